# revision 1
# baseline (speedup 1.0000x reference)
"""Trainium2 Bass kernel for nn_ConnectionTransformer (8 NeuronCores, SPMD).

Strategy
--------
- Phase A (embed + compress attention): batch-parallel, core c handles batch c.
- Phase B (6 bilinear message-passing steps): target-slot sharding — core c owns
  16 target slots j in [16c, 16c+16). Each core computes the full influence for
  its slots (sum over all source slots i), applies relu/residual/LayerNorm
  locally, and an AllGather rebuilds the replicated transposed state h^T each
  step. The per-pair weights W_source/W_target (1 GB total) are sharded along j
  and streamed from HBM once per step per core (64+64 MB) — the memory roofline.
- Phase C (expand attention + vocab projection): batch-parallel again.

All weights are pre-transposed/tiled on the host into matmul-ready layouts so
the device never transposes weight tensors.
"""
import os
import sys

sys.path.insert(0, "/opt/trn_rl_repo")

import numpy as np
from concourse import bass, bacc, tile, bass_utils, mybir
from concourse import masks

B, L, D, S, R, STEPS, V = 8, 512, 256, 128, 32, 6, 32000
NC = 8
JL = S // NC          # 16 local target slots per core
K = JL // 4           # 4 quads of target slots
VT = 500              # vocab tile width
NVT = V // VT         # 64 vocab tiles
SCALE = 1.0 / np.sqrt(D)
LN_EPS = 1e-5

F32 = mybir.dt.float32
F32R = mybir.dt.float32r
I16 = mybir.dt.int16

# "f32" (exact) or "f32r" (full-rate fp32 on the PE for the big einsums)
MM_MODE = os.environ.get("MM_MODE", "f32")
N_STEPS = int(os.environ.get("N_STEPS", str(STEPS)))


def _mm(ap):
    """Bitcast an AP to float32r when running the big einsums in f32r mode."""
    if MM_MODE == "f32r":
        return ap.bitcast(F32R)
    return ap


# ---------------------------------------------------------------------------
# Device program
# ---------------------------------------------------------------------------

def build():
    nc = bacc.Bacc("TRN2", target_bir_lowering=False, debug=False, num_devices=NC)

    io = {}

    def inp(name, shape, dtype=F32):
        io[name] = nc.dram_tensor(name, shape, dtype, kind="ExternalInput").ap()

    inp("xT_in", [D, L])
    inp("maskw", [128, 4])
    for w in ("wqT", "wkslT", "wvT", "wqoT", "wkfT", "wvfT"):
        inp(w, [D, D])
    inp("hT_in", [D, S])
    inp("hn_in", [S, D])
    inp("lngb", [B, STEPS, 2 * D])
    inp("wsrc", [S, K, 2, 128, 4 * R])
    inp("wtgt", [K, 32, 4 * R, 4, D])
    inp("woutT", [NVT, 2, 128, VT])
    io["lg_out"] = nc.dram_tensor(
        "lg_out", [L, V], F32, kind="ExternalOutput"
    ).ap()
    io["dbg"] = nc.dram_tensor(
        "dbg", [128, 2048], F32, kind="ExternalOutput"
    ).ap()

    with tile.TileContext(nc) as tc:
        _body(nc, tc, io)
    nc.compile()
    return nc


def _body(nc, tc, io):
    with tc.tile_pool(name="const", bufs=1) as const, \
         tc.tile_pool(name="state", bufs=1) as state:

        ident = const.tile([128, 128], F32)
        masks.make_identity(nc, ident[:])
        ones = const.tile([128, 1], F32)
        nc.vector.memset(ones[:], 1.0)
        eps_sb = const.tile([128, 1], F32)
        nc.vector.memset(eps_sb[:], LN_EPS)

        pid = nc.sync.partition_id()

        # persistent state
        hT = [state.tile([128, S, B], F32, name=f"hT{dt}") for dt in range(2)]
        h_upd = state.tile([B, JL, D], F32)      # this core's 16 slots, all batches
        qoT = [state.tile([128, L], F32, name=f"qoT{pt}") for pt in range(2)]
        lngb_sb = state.tile([B, STEPS, 2 * D], F32)
        nc.sync.dma_start(lngb_sb[:], io["lngb"])

        _phase_a(nc, tc, io, ident, ones, pid, hT, h_upd, qoT)
        for t in range(N_STEPS):
            _step(nc, tc, t, io["wsrc"], io["wtgt"], hT, h_upd, lngb_sb, ident,
                  eps_sb)
        _phase_c(nc, tc, io, ident, pid, hT, qoT)


def _phase_a(nc, tc, io, ident, ones, pid, hT, h_upd, qoT):
    with tc.tile_pool(name="pa_sb", bufs=1) as pa, \
         tc.tile_pool(name="pa_ps", bufs=3, space="PSUM") as pps, \
         tc.tile_pool(name="pa_tp", bufs=2, space="PSUM") as tps, \
         tc.tile_pool(name="pa_acc", bufs=1, space="PSUM") as aps, \
         tc.tile_pool(name="dram_a", bufs=1, space="DRAM") as dra:

        mask_sb = pa.tile([128, 4], F32)
        nc.sync.dma_start(mask_sb[:], io["maskw"])

        # X^T tiles [d128, t512] (host-gathered embeddings, transposed)
        xT = [pa.tile([128, L], F32, name=f"xT{ct}") for ct in range(2)]
        for ct in range(2):
            nc.sync.dma_start(xT[ct][:], io["xT_in"][128 * ct : 128 * (ct + 1), :])

        # weight tiles [d128, 256] (contraction on partitions)
        def load_w(name):
            ts = [pa.tile([128, D], F32, name=f"{name}_{ct}") for ct in range(2)]
            for ct in range(2):
                nc.sync.dma_start(ts[ct][:], io[name][128 * ct : 128 * (ct + 1), :])
            return ts

        wq_sb = load_w("wqT")
        wv_sb = load_w("wvT")
        wksl_sb = load_w("wkslT")
        wqo_sb = load_w("wqoT")
        hTt = [pa.tile([128, S], F32, name=f"hTt{ct}") for ct in range(2)]
        for ct in range(2):
            nc.sync.dma_start(hTt[ct][:], io["hT_in"][128 * ct : 128 * (ct + 1), :])
        hn_sb = pa.tile([S, D], F32)
        nc.sync.dma_start(hn_sb[:], io["hn_in"])

        # Q_in^T and Q_out^T : [d'128 x 2, t512]
        qT = [pa.tile([128, L], F32, name=f"qT{pt}") for pt in range(2)]
        for pt in range(2):
            for dst, wsb in ((qT, wq_sb), (qoT, wqo_sb)):
                ps = pps.tile([128, L], F32, tag="ps")
                for ct in range(2):
                    nc.tensor.matmul(
                        ps[:], wsb[ct][:, 128 * pt : 128 * (pt + 1)], xT[ct][:],
                        start=(ct == 0), stop=(ct == 1),
                    )
                nc.vector.tensor_copy(dst[pt][:], ps[:])

        # V_in natural [t128 x 4, d256]
        vn = pa.tile([128, 4, D], F32)
        for tt in range(4):
            ps = pps.tile([128, L], F32, tag="ps")
            for ct in range(2):
                nc.tensor.matmul(
                    ps[:, 0:D], xT[ct][:, 128 * tt : 128 * (tt + 1)], wv_sb[ct][:],
                    start=(ct == 0), stop=(ct == 1),
                )
            nc.vector.tensor_copy(vn[:, tt, :], ps[:, 0:D])

        # K_slots^T [d'128 x 2, s128]
        kslT = [pa.tile([128, S], F32, name=f"kslT{pt}") for pt in range(2)]
        for pt in range(2):
            ps = pps.tile([128, L], F32, tag="ps")
            for ct in range(2):
                nc.tensor.matmul(
                    ps[:, 0:S], wksl_sb[ct][:, 128 * pt : 128 * (pt + 1)], hTt[ct][:],
                    start=(ct == 0), stop=(ct == 1),
                )
            nc.vector.tensor_copy(kslT[pt][:], ps[:, 0:S])

        # attention scores + masked softmax
        a_sb = pa.tile([128, 4, S], F32)
        for tt in range(4):
            sc = pps.tile([128, L], F32, tag="ps")
            for pt in range(2):
                nc.tensor.matmul(
                    sc[:, 0:S], qT[pt][:, 128 * tt : 128 * (tt + 1)], kslT[pt][:],
                    start=(pt == 0), stop=(pt == 1),
                )
            rowmax = pa.tile([128, 1], F32, tag="rmax")
            nc.vector.tensor_reduce(
                rowmax[:], sc[:, 0:S], axis=mybir.AxisListType.X,
                op=mybir.AluOpType.max,
            )
            nb = pa.tile([128, 1], F32, tag="nb")
            nc.vector.tensor_scalar_mul(nb[:], rowmax[:], -SCALE)
            sumexp = pa.tile([128, 1], F32, tag="sexp")
            nc.scalar.activation(
                a_sb[:, tt, :], sc[:, 0:S], mybir.ActivationFunctionType.Exp,
                bias=nb[:], scale=SCALE, accum_out=sumexp[:],
            )
            rs = pa.tile([128, 1], F32, tag="rs")
            nc.vector.reciprocal(rs[:], sumexp[:])
            rm = pa.tile([128, 1], F32, tag="rmk")
            nc.vector.tensor_tensor(
                rm[:], rs[:], mask_sb[:, tt : tt + 1], op=mybir.AluOpType.mult
            )
            nc.vector.tensor_scalar_mul(a_sb[:, tt, :], a_sb[:, tt, :], rm[:])

        # column sums and IR = A^T @ V
        cs = aps.tile([128, 1], F32, tag="cs")
        for tt in range(4):
            nc.tensor.matmul(
                cs[:], a_sb[:, tt, :], ones[:, 0:1], start=(tt == 0), stop=(tt == 3)
            )
        ir = aps.tile([128, D], F32, tag="ir")
        for tt in range(4):
            nc.tensor.matmul(
                ir[:], a_sb[:, tt, :], vn[:, tt, :], start=(tt == 0), stop=(tt == 3)
            )
        cssb = pa.tile([128, 1], F32)
        nc.vector.tensor_scalar_add(cssb[:], cs[:], 1e-8)
        rcs = pa.tile([128, 1], F32)
        nc.vector.reciprocal(rcs[:], cssb[:])
        h0 = pa.tile([S, D], F32)
        nc.vector.scalar_tensor_tensor(
            h0[:], ir[:], rcs[:], hn_sb[:],
            op0=mybir.AluOpType.mult, op1=mybir.AluOpType.add,
        )

        nc.sync.dma_start(io["dbg"][:, 1024:1280], h0[:])
        # h0 -> transposed bounce + natural bounce, init AllGather
        agin0 = dra.tile([4 * 16384], F32)
        for dt in range(2):
            p3 = tps.tile([128, 128], F32, tag="tp")
            nc.tensor.transpose(p3[:], h0[:, 128 * dt : 128 * (dt + 1)], ident[:])
            h0T = pa.tile([128, 128], F32, tag="h0T")
            nc.vector.tensor_copy(h0T[:], p3[:])
            nc.sync.dma_start(
                agin0[dt * 16384 : (dt + 1) * 16384].rearrange(
                    "(p f) -> p f", p=128
                ),
                h0T[:],
            )
        nc.sync.dma_start(
            agin0[32768:65536].rearrange("(p f) -> p f", p=128), h0[:]
        )
        agout0 = dra.tile([NC, 4 * 16384], F32, addr_space="Shared")
        nc.gpsimd.collective_compute(
            "AllGather", mybir.AluOpType.bypass,
            ins=[agin0[:].opt()], outs=[agout0[:].opt()],
            replica_groups=[list(range(NC))],
        )
        # readback: hT[dt][dp, s, b] ; h_upd[b, jl, d] (own slot range via pid)
        ag0r = agout0[:].rearrange(
            "b (seg dp s) -> seg dp s b", seg=4, dp=128, s=128
        )
        for dt in range(2):
            nc.sync.dma_start(hT[dt][:], ag0r[dt])
        nc.sync.dma_start(
            h_upd[:].rearrange("b jl d -> b (jl d)"),
            agout0[:][:, bass.ds(pid * (JL * D) + 32768, JL * D)],
        )


def _step(nc, tc, t, wsrc, wtgt, hT, h_upd, lngb_sb, ident, eps_sb):
    """One message-passing step."""
    with tc.tile_pool(name=f"s{t}_ws", bufs=4) as wsp, \
         tc.tile_pool(name=f"s{t}_wt", bufs=6) as wtp, \
         tc.tile_pool(name=f"s{t}_sb", bufs=1) as sb, \
         tc.tile_pool(name=f"s{t}_p1", bufs=2, space="PSUM") as p1p, \
         tc.tile_pool(name=f"s{t}_p2", bufs=1, space="PSUM") as p2p, \
         tc.tile_pool(name=f"s{t}_p3", bufs=2, space="PSUM") as p3p, \
         tc.tile_pool(name=f"s{t}_dram", bufs=1, space="DRAM") as drp:

        inter = sb.tile([128, K, S, B], F32)

        # ---- einsum1: inter[(jloc,r), k, i, b] = h[b,i,:] @ W_source[i, j] ----
        for i in range(S):
            ws = wsp.tile([128, K, 2, 4 * R], F32, tag="ws")
            nc.sync.dma_start(
                ws[:], wsrc[i].rearrange("k dt dp jr -> dp k dt jr")
            )
            p1 = p1p.tile([128, K, B], F32, tag="p1")
            for k in range(K):
                for dt in range(2):
                    nc.tensor.matmul(
                        p1[:, k, :],
                        _mm(ws[:, k, dt, :]),
                        _mm(hT[dt][:, i, :]),
                        start=(dt == 0), stop=(dt == 1),
                    )
            nc.vector.tensor_copy(inter[:, :, i, :], p1[:])

        # ---- einsum2 + relu/residual/LN per quad k ----
        hTloc = [sb.tile([128, JL * B], F32, name=f"hTl{dt}") for dt in range(2)]
        for k in range(K):
            p2 = [p2p.tile([B, D], F32, tag=f"p2_{jl}", name=f"p2_{jl}") for jl in range(4)]
            for g in range(32):
                wt = wtp.tile([128, 4, D], F32, tag="wt")
                nc.sync.dma_start(wt[:], wtgt[k, g])
                for il in range(4):
                    i = 4 * g + il
                    for jloc in range(4):
                        nc.tensor.matmul(
                            p2[jloc][:],
                            _mm(inter[32 * jloc : 32 * (jloc + 1), k, i, :]),
                            _mm(wt[32 * jloc : 32 * (jloc + 1), il, :]),
                            start=(i == 0), stop=(i == S - 1),
                            tile_position=(32 * jloc, 0),
                        )
            # relu, +h_old, LayerNorm over d for the 4 slots of this quad
            hrelu = sb.tile([B, 4, D], F32, tag="hrelu")
            for jloc in range(4):
                nc.scalar.activation(
                    hrelu[:, jloc, :], p2[jloc][:],
                    mybir.ActivationFunctionType.Relu,
                )
            hsum = sb.tile([B, 4, D], F32, tag="hsum")
            nc.vector.tensor_tensor(
                hsum[:], hrelu[:], h_upd[:, 4 * k : 4 * (k + 1), :],
                op=mybir.AluOpType.add,
            )
            mean = sb.tile([B, 4], F32, tag="mean")
            nc.vector.tensor_reduce(
                mean[:], hsum[:], axis=mybir.AxisListType.X, op=mybir.AluOpType.add
            )
            nc.vector.tensor_scalar_mul(mean[:], mean[:], 1.0 / D)
            cen = sb.tile([B, 4, D], F32, tag="cen")
            nc.vector.tensor_tensor(
                cen[:], hsum[:], mean[:].to_broadcast((B, 4, D)),
                op=mybir.AluOpType.subtract,
            )
            sq = sb.tile([B, 4, D], F32, tag="sq")
            nc.vector.tensor_tensor(
                sq[:], cen[:], cen[:], op=mybir.AluOpType.mult
            )
            var = sb.tile([B, 4], F32, tag="var")
            nc.vector.tensor_reduce(
                var[:], sq[:], axis=mybir.AxisListType.X, op=mybir.AluOpType.add
            )
            std = sb.tile([B, 4], F32, tag="std")
            nc.scalar.activation(
                std[:], var[:], mybir.ActivationFunctionType.Sqrt,
                bias=eps_sb[0:B, :], scale=1.0 / D,
            )
            rstd = sb.tile([B, 4], F32, tag="rstd")
            nc.vector.reciprocal(rstd[:], std[:])
            hnorm = sb.tile([B, 4, D], F32, tag="hnorm")
            nc.vector.tensor_tensor(
                hnorm[:], cen[:], rstd[:].to_broadcast((B, 4, D)),
                op=mybir.AluOpType.mult,
            )
            g_b = lngb_sb[:, t : t + 1, 0:D].to_broadcast((B, 4, D))
            b_b = lngb_sb[:, t : t + 1, D : 2 * D].to_broadcast((B, 4, D))
            nc.vector.tensor_tensor(
                hnorm[:], hnorm[:], g_b, op=mybir.AluOpType.mult
            )
            nc.vector.tensor_tensor(
                h_upd[:, 4 * k : 4 * (k + 1), :], hnorm[:], b_b,
                op=mybir.AluOpType.add,
            )
            # transpose the 4 updated slots into hTloc
            for jloc in range(4):
                jl = 4 * k + jloc
                for dt in range(2):
                    p3 = p3p.tile([128, B], F32, tag="p3")
                    nc.tensor.transpose(
                        p3[:],
                        h_upd[:, jl, 128 * dt : 128 * (dt + 1)],
                        ident[0:B, 0:B],
                    )
                    nc.vector.tensor_copy(
                        hTloc[dt][:, jl * B : (jl + 1) * B], p3[:]
                    )

        # ---- AllGather the transposed updated slots; rebuild hT ----
        agin = drp.tile([2 * 128 * JL * B], F32)
        for dt in range(2):
            nc.sync.dma_start(
                agin[dt * 16384 : (dt + 1) * 16384].rearrange(
                    "(p f) -> p f", p=128
                ),
                hTloc[dt][:],
            )
        agout = drp.tile([NC, 2 * 128 * JL * B], F32, addr_space="Shared")
        nc.gpsimd.collective_compute(
            "AllGather", mybir.AluOpType.bypass,
            ins=[agin[:].opt()], outs=[agout[:].opt()],
            replica_groups=[list(range(NC))],
        )
        agr = agout[:].rearrange(
            "rk (dt dp jl b) -> dt dp rk jl b", dt=2, dp=128, jl=JL, b=B
        )
        for dt in range(2):
            nc.sync.dma_start(
                hT[dt][:].rearrange("dp (rk jl) b -> dp rk jl b", rk=NC), agr[dt]
            )


def _phase_c(nc, tc, io, ident, pid, hT, qoT):
    with tc.tile_pool(name="pc_sb", bufs=1) as pc, \
         tc.tile_pool(name="pc_ps", bufs=3, space="PSUM") as cps, \
         tc.tile_pool(name="pc_lg", bufs=4, space="PSUM") as lgps, \
         tc.tile_pool(name="pc_wo", bufs=4) as wop:

        wkf_sb = [pc.tile([128, D], F32, name=f"wkf{ct}") for ct in range(2)]
        wvf_sb = [pc.tile([128, D], F32, name=f"wvf{ct}") for ct in range(2)]
        for ct in range(2):
            nc.sync.dma_start(
                wkf_sb[ct][:], io["wkfT"][128 * ct : 128 * (ct + 1), :]
            )
            nc.sync.dma_start(
                wvf_sb[ct][:], io["wvfT"][128 * ct : 128 * (ct + 1), :]
            )

        # own-batch h^T slice (dynamic b=pid) -> static tiles
        pid_v = nc.vector.partition_id()
        hb = [pc.tile([128, S], F32, name=f"hb{dt}") for dt in range(2)]
        for dt in range(2):
            nc.vector.tensor_copy(
                hb[dt][:].rearrange("p (s o) -> p s o", o=1),
                hT[dt][:, :, bass.ds(pid_v, 1)],
            )

        nc.sync.dma_start(io["dbg"][:, 1280:1408], hb[0][:])
        nc.sync.dma_start(io["dbg"][:, 1408:1536], hb[1][:])
        # K_f^T [d'128 x2, s128] ; V_f natural [s, d']
        kfT = [pc.tile([128, S], F32, name=f"kfT{pt}") for pt in range(2)]
        for pt in range(2):
            ps = cps.tile([128, L], F32, tag="c")
            for ct in range(2):
                nc.tensor.matmul(
                    ps[:, 0:S], wkf_sb[ct][:, 128 * pt : 128 * (pt + 1)], hb[ct][:],
                    start=(ct == 0), stop=(ct == 1),
                )
            nc.vector.tensor_copy(kfT[pt][:], ps[:, 0:S])
        vf = pc.tile([S, D], F32)
        psv = cps.tile([128, L], F32, tag="c")
        for ct in range(2):
            nc.tensor.matmul(
                psv[0:S, 0:D], hb[ct][:], wvf_sb[ct][:],
                start=(ct == 0), stop=(ct == 1),
            )
        nc.vector.tensor_copy(vf[:], psv[0:S, 0:D])

        # expand attention -> A2^T [s, t512]
        a2T = pc.tile([S, L], F32)
        for tt in range(4):
            sc = cps.tile([128, L], F32, tag="c")
            for pt in range(2):
                nc.tensor.matmul(
                    sc[:, 0:S], qoT[pt][:, 128 * tt : 128 * (tt + 1)], kfT[pt][:],
                    start=(pt == 0), stop=(pt == 1),
                )
            rowmax = pc.tile([128, 1], F32, tag="rmax2")
            nc.vector.tensor_reduce(
                rowmax[:], sc[:, 0:S], axis=mybir.AxisListType.X,
                op=mybir.AluOpType.max,
            )
            nb = pc.tile([128, 1], F32, tag="nb2")
            nc.vector.tensor_scalar_mul(nb[:], rowmax[:], -SCALE)
            a2 = pc.tile([128, S], F32, tag="a2")
            sumexp = pc.tile([128, 1], F32, tag="sexp2")
            nc.scalar.activation(
                a2[:], sc[:, 0:S], mybir.ActivationFunctionType.Exp,
                bias=nb[:], scale=SCALE, accum_out=sumexp[:],
            )
            rs = pc.tile([128, 1], F32, tag="rs2")
            nc.vector.reciprocal(rs[:], sumexp[:])
            nc.vector.tensor_scalar_mul(a2[:], a2[:], rs[:])
            ptr = cps.tile([128, L], F32, tag="c")
            nc.tensor.transpose(ptr[:, 0:S], a2[:], ident[:])
            nc.vector.tensor_copy(a2T[:, 128 * tt : 128 * (tt + 1)], ptr[:, 0:S])

        # Y^T [d128 x2, t512]
        yT = [pc.tile([128, L], F32, name=f"yT{dt}") for dt in range(2)]
        for dt in range(2):
            ps = cps.tile([128, L], F32, tag="c")
            nc.tensor.matmul(
                ps[:], vf[:, 128 * dt : 128 * (dt + 1)], a2T[:],
                start=True, stop=True,
            )
            nc.vector.tensor_copy(yT[dt][:], ps[:])

        nc.sync.dma_start(io["dbg"][:, 1536:2048], yT[0][:])
        # logits tiles + direct PSUM->DRAM store
        for vt in range(NVT):
            wo_sb = wop.tile([128, 2, VT], F32, tag="wo")
            nc.sync.dma_start(
                wo_sb[:], io["woutT"][vt].rearrange("dt dp v -> dp dt v")
            )
            for tt in range(4):
                lg = lgps.tile([128, VT], F32, tag="lg")
                for dt in range(2):
                    nc.tensor.matmul(
                        lg[:],
                        _mm(yT[dt][:, 128 * tt : 128 * (tt + 1)]),
                        _mm(wo_sb[:, dt, :]),
                        start=(dt == 0), stop=(dt == 1),
                    )
                lg_sb = wop.tile([128, VT], F32, tag="lg_sb", name="lg_sb")
                nc.any.tensor_copy(lg_sb[:], lg[:])
                nc.sync.dma_start(
                    io["lg_out"][
                        128 * tt : 128 * (tt + 1), VT * vt : VT * (vt + 1)
                    ],
                    lg_sb[:],
                )


# ---------------------------------------------------------------------------
# Host side
# ---------------------------------------------------------------------------

_NC_CACHE = {}


def _get_nc():
    key = (MM_MODE, N_STEPS)
    if key not in _NC_CACHE:
        _NC_CACHE[key] = build()
    return _NC_CACHE[key]


def _prep_in_maps(inputs):
    f32 = lambda a: np.ascontiguousarray(np.asarray(a), dtype=np.float32)
    input_ids = np.asarray(inputs["input_ids"])
    attention_mask = np.asarray(inputs["attention_mask"])
    H = f32(inputs["H"])
    W_source = f32(inputs["W_source"])
    W_target = f32(inputs["W_target"])

    lngb = np.zeros((B, STEPS, 2 * D), dtype=np.float32)
    lngb[:, :, 0:D] = np.asarray(inputs["ln_scale"])[None]
    lngb[:, :, D:] = np.asarray(inputs["ln_bias"])[None]

    rep = {
        "wqT": f32(np.asarray(inputs["Wq_in"]).T),
        "wkslT": f32(np.asarray(inputs["Wk_slots"]).T),
        "wvT": f32(np.asarray(inputs["Wv_in"]).T),
        "wqoT": f32(np.asarray(inputs["Wq_out"]).T),
        "wkfT": f32(np.asarray(inputs["Wk_fin"]).T),
        "wvfT": f32(np.asarray(inputs["Wv_fin"]).T),
        "hT_in": f32(H.T),
        "hn_in": H,
        "lngb": lngb,
        # woutT[vt, dtile, dp, vl] = Wout[500vt+vl, 128dt+dp]
        "woutT": np.ascontiguousarray(
            f32(inputs["W_out_proj"]).reshape(NVT, VT, 2, 128).transpose(0, 2, 3, 1)
        ),
    }

    in_maps = []
    for c in range(NC):
        m = dict(rep)
        X = (np.asarray(inputs["token_emb"], dtype=np.float32)[input_ids[c]]
             + np.asarray(inputs["pos_emb"], dtype=np.float32))
        m["xT_in"] = np.ascontiguousarray(X.T)
        m["maskw"] = np.ascontiguousarray(
            attention_mask[c].astype(np.float32).reshape(4, 128).T
        )
        # wsrc[i, k, dt, dp, (jloc r)] = W_source[i, 16c+4k+jloc, 128dt+dp, r]
        ws = W_source[:, JL * c : JL * (c + 1)]      # [S, 16, D, R]
        ws = ws.reshape(S, K, 4, 2, 128, R).transpose(0, 1, 3, 4, 2, 5)
        m["wsrc"] = np.ascontiguousarray(ws).reshape(S, K, 2, 128, 4 * R)
        # wtgt[k, g, (jloc r), il, d] = W_target[4g+il, 16c+4k+jloc, r, d]
        # The reference masks out the i == j (diagonal) pair; zeroing
        # W_target[j, j] is exactly equivalent since the term is linear in it.
        wt = W_target[:, JL * c : JL * (c + 1)].copy()   # [S, 16, R, D]
        for jl in range(JL):
            wt[JL * c + jl, jl] = 0.0
        wt = wt.reshape(32, 4, K, 4, R, D).transpose(2, 0, 3, 4, 1, 5)
        m["wtgt"] = np.ascontiguousarray(wt).reshape(K, 32, 4 * R, 4, D)
        in_maps.append(m)
    return in_maps


def run(inputs, trace=False):
    nc = _get_nc()
    in_maps = _prep_in_maps(inputs)
    res = bass_utils.run_bass_kernel_spmd(
        nc, in_maps, core_ids=list(range(NC)), trace=trace
    )
    out = np.stack([res.results[c]["lg_out"] for c in range(NC)], axis=0)
    return out, res


def kernel(**inputs):
    out, _ = run(inputs, trace=False)
    return out



# revision 9
# speedup vs baseline: 2.5152x; 2.5152x over previous
"""Trainium2 Bass kernel for nn_ConnectionTransformer (8 NeuronCores, SPMD).

Strategy (v2)
-------------
- Phase A (embed + compress attention): batch-parallel, core c handles batch c.
  fp32 math; produces the replicated transposed slot state.
- Phase B (6 bilinear message-passing steps): target-slot sharding - core c owns
  16 target slots j. Per-pair weights are cast to fp16 on the host (rel err
  ~5e-4 on the final logits, far under the 2e-2 gate) which halves the HBM
  stream (67 MB/core/step) AND makes every matmul a 1-cycle/row fp16 op with
  FWL weight loads. Both einsums are arranged so the streamed weight tile is
  the 128x128 stationary operand and the B=8 batch is the moving operand
  (8-col streams), so the PE cost is LDWEIGHTS-bound at ~55 us/step/einsum -
  under the ~190 us/step DMA floor. The step is therefore memory-bound.
- einsum1 emits inter[(jloc,r), g, il, jq, b]; an SBUF->SBUF DMA regroups it
  to [(il,r), g, jloc, jq, b] so einsum2 can contract (il,r)=128 per
  (j, g, dh) with full-height stationary tiles.
- relu/residual/LayerNorm run entirely in the transposed (d-on-partitions)
  layout: partition-dim sums via ones-matmuls, per-(j,b) mean/rstd broadcast
  back with a 1-row ones matmul, affine via tensor_scalar with per-partition
  gamma/beta. Output is already the hT layout the next step needs - no
  per-slot PE transposes.
- Each step AllGathers the 16 updated slots in fp16 (64 KB/core); the
  residual path keeps the core's own slots in fp32 locally.
- Phase C (expand attention + vocab projection): batch-parallel, fp16
  operands for the big matmuls, fp32 softmax/logits. Logits stores are
  batched per vocab tile ([128,4,500] = 1 MB per DMA).

Queueing: weight streams ride the SP (wsrc) and Activation (wtgt) HWDGE
queues so the two streams self-sequence; collective bounces ride the Pool
SWDGE queue so they never block next-step weight prefetch.
"""
import os
import sys

sys.path.insert(0, "/opt/trn_rl_repo")

import numpy as np
from concourse import bass, bacc, tile, bass_utils, mybir
from concourse import masks

B, L, D, S, R, STEPS, V = 8, 512, 256, 128, 32, 6, 32000
NC = 8
JL = S // NC          # 16 local target slots per core
G = S // 4            # 32 chunks of 4 source slots
VT = 500              # vocab tile width
NVT = V // VT         # 64 vocab tiles
SCALE = 1.0 / np.sqrt(D)
LN_EPS = 1e-5

F32 = mybir.dt.float32
F16 = mybir.dt.float16

N_STEPS = int(os.environ.get("N_STEPS", str(STEPS)))
DBG = bool(int(os.environ.get("DBG", "0")))


# ---------------------------------------------------------------------------
# Device program
# ---------------------------------------------------------------------------

def build():
    nc = bacc.Bacc("TRN2", target_bir_lowering=False, debug=False, num_devices=NC)

    io = {}

    def inp(name, shape, dtype=F32):
        io[name] = nc.dram_tensor(name, shape, dtype, kind="ExternalInput").ap()

    inp("xT_in", [D, L])
    inp("maskw", [128, 4])
    for w in ("wqT", "wkslT", "wvT", "wqoT"):
        inp(w, [D, D])
    inp("wkf16", [D, D], F16)
    inp("wvf16", [D, D], F16)
    inp("hT_in", [D, S])
    inp("hn_in", [S, D])
    inp("lngbT", [128, STEPS, 4])
    inp("wsrc16", [G, 128, 4, 4, 2, 128], F16)
    inp("wtgt16", [G, 128, JL, 2, 128], F16)
    inp("wout16", [NVT, 128, 2, VT], F16)
    io["lg_out"] = nc.dram_tensor(
        "lg_out", [L, V], F32, kind="ExternalOutput"
    ).ap()
    if DBG:
        io["dbg"] = nc.dram_tensor(
            "dbg", [128, 4096], F32, kind="ExternalOutput"
        ).ap()

    with tile.TileContext(nc) as tc:
        _body(nc, tc, io)
    nc.compile()
    return nc


def _body(nc, tc, io):
    with tc.tile_pool(name="const", bufs=1) as const, \
         tc.tile_pool(name="state", bufs=1) as state:

        ident = const.tile([128, 128], F32)
        masks.make_identity(nc, ident[:])
        ones = const.tile([128, 1], F32)
        nc.vector.memset(ones[:], 1.0)
        ones_row = const.tile([1, 128], F32)
        nc.vector.memset(ones_row[:], 1.0)
        eps_sb = const.tile([128, 1], F32)
        nc.vector.memset(eps_sb[:], LN_EPS)

        # persistent state
        hTh = [state.tile([128, S, B], F16, name=f"hTh{dt}") for dt in range(2)]
        hTown = [state.tile([128, JL, B], F32, name=f"hTo{dt}") for dt in range(2)]
        qoT16 = [state.tile([128, L], F16, name=f"qoT{pt}") for pt in range(2)]
        lngbT_sb = state.tile([128, STEPS, 4], F32)
        nc.sync.dma_start(lngbT_sb[:], io["lngbT"])

        pid_v = nc.vector.partition_id()

        _phase_a(nc, tc, io, ident, ones, pid_v, hTh, hTown, qoT16)
        with tc.tile_pool(name="ws", bufs=3) as wsp, \
             tc.tile_pool(name="wt", bufs=3) as wtp, \
             tc.tile_pool(name="istep", bufs=2) as isp, \
             tc.tile_pool(name="lnsb", bufs=2) as lnsb, \
             tc.tile_pool(name="p1", bufs=2, space="PSUM") as p1p, \
             tc.tile_pool(name="infl", bufs=1, space="PSUM") as inflp, \
             tc.tile_pool(name="lnps", bufs=1, space="PSUM") as lnps, \
             tc.tile_pool(name="bdram", bufs=2, space="DRAM") as bdram:
            for t in range(N_STEPS):
                _step(nc, tc, t, io, hTh, hTown, lngbT_sb, ones, ones_row,
                      eps_sb, wsp, wtp, isp, lnsb, p1p, inflp, lnps, bdram)
        _phase_c(nc, tc, io, ident, pid_v, hTh, qoT16)


def _phase_a(nc, tc, io, ident, ones, pid_v, hTh, hTown, qoT16):
    with tc.tile_pool(name="pa_sb", bufs=1) as pa, \
         tc.tile_pool(name="pa_ps", bufs=3, space="PSUM") as pps, \
         tc.tile_pool(name="pa_tp", bufs=2, space="PSUM") as tps, \
         tc.tile_pool(name="pa_acc", bufs=1, space="PSUM") as aps, \
         tc.tile_pool(name="dram_a", bufs=1, space="DRAM") as dra:

        mask_sb = pa.tile([128, 4], F32)
        nc.sync.dma_start(mask_sb[:], io["maskw"])

        # X^T tiles [d128, t512] (host-gathered embeddings, transposed)
        xT = [pa.tile([128, L], F32, name=f"xT{ct}") for ct in range(2)]
        for ct in range(2):
            nc.sync.dma_start(xT[ct][:], io["xT_in"][128 * ct : 128 * (ct + 1), :])

        def load_w(name):
            ts = [pa.tile([128, D], F32, name=f"{name}_{ct}") for ct in range(2)]
            for ct in range(2):
                nc.sync.dma_start(ts[ct][:], io[name][128 * ct : 128 * (ct + 1), :])
            return ts

        wq_sb = load_w("wqT")
        wv_sb = load_w("wvT")
        wksl_sb = load_w("wkslT")
        wqo_sb = load_w("wqoT")
        hTt = [pa.tile([128, S], F32, name=f"hTt{ct}") for ct in range(2)]
        for ct in range(2):
            nc.sync.dma_start(hTt[ct][:], io["hT_in"][128 * ct : 128 * (ct + 1), :])
        hn_sb = pa.tile([S, D], F32)
        nc.sync.dma_start(hn_sb[:], io["hn_in"])

        # Q_in^T and Q_out^T : [d'128 x 2, t512]
        qT = [pa.tile([128, L], F32, name=f"qT{pt}") for pt in range(2)]
        qoT = [pa.tile([128, L], F32, name=f"qoTf{pt}") for pt in range(2)]
        for pt in range(2):
            for dst, wsb in ((qT, wq_sb), (qoT, wqo_sb)):
                ps = pps.tile([128, L], F32, tag="ps")
                for ct in range(2):
                    nc.tensor.matmul(
                        ps[:], wsb[ct][:, 128 * pt : 128 * (pt + 1)], xT[ct][:],
                        start=(ct == 0), stop=(ct == 1),
                    )
                nc.vector.tensor_copy(dst[pt][:], ps[:])
            nc.vector.tensor_copy(qoT16[pt][:], qoT[pt][:])

        # V_in natural [t128 x 4, d256]
        vn = pa.tile([128, 4, D], F32)
        for tt in range(4):
            ps = pps.tile([128, L], F32, tag="ps")
            for ct in range(2):
                nc.tensor.matmul(
                    ps[:, 0:D], xT[ct][:, 128 * tt : 128 * (tt + 1)], wv_sb[ct][:],
                    start=(ct == 0), stop=(ct == 1),
                )
            nc.vector.tensor_copy(vn[:, tt, :], ps[:, 0:D])

        # K_slots^T [d'128 x 2, s128]
        kslT = [pa.tile([128, S], F32, name=f"kslT{pt}") for pt in range(2)]
        for pt in range(2):
            ps = pps.tile([128, L], F32, tag="ps")
            for ct in range(2):
                nc.tensor.matmul(
                    ps[:, 0:S], wksl_sb[ct][:, 128 * pt : 128 * (pt + 1)], hTt[ct][:],
                    start=(ct == 0), stop=(ct == 1),
                )
            nc.vector.tensor_copy(kslT[pt][:], ps[:, 0:S])

        # attention scores + masked softmax
        a_sb = pa.tile([128, 4, S], F32)
        for tt in range(4):
            sc = pps.tile([128, L], F32, tag="ps")
            for pt in range(2):
                nc.tensor.matmul(
                    sc[:, 0:S], qT[pt][:, 128 * tt : 128 * (tt + 1)], kslT[pt][:],
                    start=(pt == 0), stop=(pt == 1),
                )
            rowmax = pa.tile([128, 1], F32, tag="rmax")
            nc.vector.tensor_reduce(
                rowmax[:], sc[:, 0:S], axis=mybir.AxisListType.X,
                op=mybir.AluOpType.max,
            )
            nb = pa.tile([128, 1], F32, tag="nb")
            nc.vector.tensor_scalar_mul(nb[:], rowmax[:], -SCALE)
            sumexp = pa.tile([128, 1], F32, tag="sexp")
            nc.scalar.activation(
                a_sb[:, tt, :], sc[:, 0:S], mybir.ActivationFunctionType.Exp,
                bias=nb[:], scale=SCALE, accum_out=sumexp[:],
            )
            rs = pa.tile([128, 1], F32, tag="rs")
            nc.vector.reciprocal(rs[:], sumexp[:])
            rm = pa.tile([128, 1], F32, tag="rmk")
            nc.vector.tensor_tensor(
                rm[:], rs[:], mask_sb[:, tt : tt + 1], op=mybir.AluOpType.mult
            )
            nc.vector.tensor_scalar_mul(a_sb[:, tt, :], a_sb[:, tt, :], rm[:])

        # column sums and IR = A^T @ V
        cs = aps.tile([128, 1], F32, tag="cs")
        for tt in range(4):
            nc.tensor.matmul(
                cs[:], a_sb[:, tt, :], ones[:, 0:1], start=(tt == 0), stop=(tt == 3)
            )
        ir = aps.tile([128, D], F32, tag="ir")
        for tt in range(4):
            nc.tensor.matmul(
                ir[:], a_sb[:, tt, :], vn[:, tt, :], start=(tt == 0), stop=(tt == 3)
            )
        cssb = pa.tile([128, 1], F32)
        nc.vector.tensor_scalar_add(cssb[:], cs[:], 1e-8)
        rcs = pa.tile([128, 1], F32)
        nc.vector.reciprocal(rcs[:], cssb[:])
        h0 = pa.tile([S, D], F32)
        nc.vector.scalar_tensor_tensor(
            h0[:], ir[:], rcs[:], hn_sb[:],
            op0=mybir.AluOpType.mult, op1=mybir.AluOpType.add,
        )

        # h0 -> transposed bounce, init AllGather (fp32, 2 segments)
        agin0 = dra.tile([2 * 16384], F32)
        for dt in range(2):
            p3 = tps.tile([128, 128], F32, tag="tp")
            nc.tensor.transpose(p3[:], h0[:, 128 * dt : 128 * (dt + 1)], ident[:])
            h0T = pa.tile([128, 128], F32, tag="h0T")
            nc.vector.tensor_copy(h0T[:], p3[:])
            nc.sync.dma_start(
                agin0[dt * 16384 : (dt + 1) * 16384].rearrange(
                    "(p f) -> p f", p=128
                ),
                h0T[:],
            )
        agout0 = dra.tile([NC, 2 * 16384], F32, addr_space="Shared")
        nc.gpsimd.collective_compute(
            "AllGather", mybir.AluOpType.bypass,
            ins=[agin0[:].opt()], outs=[agout0[:].opt()],
            replica_groups=[list(range(NC))],
        )
        # readback: full transposed state -> cast fp16 + own-slot fp32
        ag0r = agout0[:].rearrange(
            "b (seg dp s) -> seg dp s b", seg=2, dp=128, s=128
        )
        hT32 = [pa.tile([128, S, B], F32, name=f"hT32_{dt}") for dt in range(2)]
        for dt in range(2):
            nc.sync.dma_start(hT32[dt][:], ag0r[dt])
            nc.vector.tensor_copy(hTh[dt][:], hT32[dt][:])
            nc.vector.tensor_copy(
                hTown[dt][:], hT32[dt][:, bass.ds(pid_v * JL, JL), :]
            )


def _step(nc, tc, t, io, hTh, hTown, lngbT_sb, ones, ones_row, eps_sb,
          wsp, wtp, isp, lnsb, p1p, inflp, lnps, bdram):
    """One message-passing step (fp16 weights, transposed-layout LN)."""
    # whole-step inter buffers: [p, g, il, jq, b] and regrouped [p, g, jloc, jq, b]
    inter = isp.tile([128, G, 4, 4, B], F16, tag="inter")
    inter2 = isp.tile([128, G, 4, 4, B], F16, tag="inter2")
    inflT = inflp.tile([128, 2, JL, B], F32, tag="inflT")

    # ---- einsum1: inter[(jloc,r), g, il, jq, b] ----
    for g in range(G):
        ws = wsp.tile([128, 4, 4, 2, 128], F16, tag="ws")
        nc.sync.dma_start(ws[:], io["wsrc16"][g])
        p1 = p1p.tile([128, 4, 4, B], F32, tag="p1")
        for il in range(4):
            i = 4 * g + il
            for jq in range(4):
                for dt in range(2):
                    nc.tensor.matmul(
                        p1[:, il, jq, :],
                        ws[:, il, jq, dt, :],
                        hTh[dt][:, i, :],
                        start=(dt == 0), stop=(dt == 1),
                    )
        nc.vector.tensor_copy(inter[:, g, :, :, :], p1[:])

    # ---- regroup: (jloc,r) bands -> (il,r) bands (SBUF->SBUF DMA) ----
    for il in range(4):
        for jloc in range(4):
            nc.sync.dma_start(
                inter2[32 * il : 32 * (il + 1), :, jloc, :, :],
                inter[32 * jloc : 32 * (jloc + 1), :, il, :, :],
            )

    # ---- einsum2: inflT[dc, dh, j, b] += inter2 @ W_target ----
    # PSUM start_tensor_calc marks the whole 2KB zero region (= the bank
    # holding all 32 (j, dh) accumulators) pending-zero, so exactly ONE
    # start on the first matmul and ONE stop on the last - a per-group
    # start would wipe the other groups' partial sums.
    for g in range(G):
        wt = wtp.tile([128, JL, 2, 128], F16, tag="wt")
        nc.scalar.dma_start(wt[:], io["wtgt16"][g])
        for j in range(JL):
            jq, jloc = j // 4, j % 4
            for dh in range(2):
                nc.tensor.matmul(
                    inflT[:, dh, j, :],
                    wt[:, j, dh, :],
                    inter2[:, g, jloc, jq, :],
                    start=(g == 0 and j == 0 and dh == 0),
                    stop=(g == G - 1 and j == JL - 1 and dh == 1),
                    skip_group_check=True,
                )

    # ---- relu + residual + LayerNorm, all in transposed layout ----
    hrelu = lnsb.tile([128, 2, JL, B], F32, tag="hrelu")
    nc.scalar.activation(hrelu[:], inflT[:], mybir.ActivationFunctionType.Relu)
    hsum = lnsb.tile([128, 2, JL, B], F32, tag="hsum")
    for dt in range(2):
        nc.vector.tensor_tensor(
            hsum[:, dt], hrelu[:, dt], hTown[dt][:], op=mybir.AluOpType.add
        )
    sq = lnsb.tile([128, 2, JL, B], F32, tag="sq")
    nc.vector.tensor_tensor(sq[:], hsum[:], hsum[:], op=mybir.AluOpType.mult)
    sums = lnps.tile([1, 2, 2, JL, B], F32, tag="sums")  # [1, (s/sq), dt, j, b]
    nc.tensor.matmul(
        sums[0:1, 0], ones[:, 0:1], hsum[:], start=True, stop=True,
    )
    nc.tensor.matmul(
        sums[0:1, 1], ones[:, 0:1], sq[:], start=True, stop=True,
    )
    # mean/rstd per (j, b): combine dt halves on 1 partition
    sums_sb = lnsb.tile([1, 2, 2, JL, B], F32, tag="sums_sb")
    nc.vector.tensor_copy(sums_sb[:], sums[:])
    mrs = lnsb.tile([1, 2, JL * B], F32, tag="mrs")  # [1, (mean, rstd), jb]
    mean = mrs[0:1, 0]
    nc.vector.tensor_tensor(
        mean, sums_sb[0:1, 0, 0].rearrange("p j b -> p (j b)"),
        sums_sb[0:1, 0, 1].rearrange("p j b -> p (j b)"), op=mybir.AluOpType.add,
    )
    nc.vector.tensor_scalar_mul(mean, mean, 1.0 / D)
    ssq = lnsb.tile([1, JL * B], F32, tag="ssq")
    nc.vector.tensor_tensor(
        ssq[:], sums_sb[0:1, 1, 0].rearrange("p j b -> p (j b)"),
        sums_sb[0:1, 1, 1].rearrange("p j b -> p (j b)"), op=mybir.AluOpType.add,
    )
    nc.vector.tensor_scalar_mul(ssq[:], ssq[:], 1.0 / D)
    msq = lnsb.tile([1, JL * B], F32, tag="msq")
    nc.vector.tensor_tensor(msq[:], mean, mean, op=mybir.AluOpType.mult)
    var = lnsb.tile([1, JL * B], F32, tag="var")
    nc.vector.tensor_tensor(var[:], ssq[:], msq[:], op=mybir.AluOpType.subtract)
    std = lnsb.tile([1, JL * B], F32, tag="std")
    nc.scalar.activation(
        std[:], var[:], mybir.ActivationFunctionType.Sqrt, bias=eps_sb[0:1, :]
    )
    rstd = mrs[0:1, 1]
    nc.vector.reciprocal(rstd, std[:])
    # broadcast mean/rstd across partitions via 1-row ones matmul
    bc = lnps.tile([128, 2, JL * B], F32, tag="bc")
    nc.tensor.matmul(
        bc[:], ones_row[:], mrs[0:1].rearrange("p m jb -> p (m jb)"),
        start=True, stop=True,
    )
    # normalize + affine; write fp32 own-state and fp16 gather input
    hnewTh = lnsb.tile([128, 2, JL, B], F16, tag="hnewTh")
    cen = lnsb.tile([128, JL, B], F32, tag="cen")
    for dt in range(2):
        nc.vector.tensor_tensor(
            cen[:], hsum[:, dt],
            bc[:, 0].rearrange("p (j b) -> p j b", j=JL),
            op=mybir.AluOpType.subtract,
        )
        nc.vector.tensor_tensor(
            cen[:], cen[:],
            bc[:, 1].rearrange("p (j b) -> p j b", j=JL),
            op=mybir.AluOpType.mult,
        )
        nc.vector.tensor_scalar(
            hTown[dt][:], cen[:],
            lngbT_sb[:, t, dt : dt + 1],
            lngbT_sb[:, t, 2 + dt : 3 + dt],
            op0=mybir.AluOpType.mult, op1=mybir.AluOpType.add,
        )
        nc.vector.tensor_copy(hnewTh[:, dt], hTown[dt][:])

    # ---- AllGather the 16 updated slots (fp16) ----
    agin = bdram.tile([128 * 2 * JL * B], F16, tag="agin")
    nc.gpsimd.dma_start(
        agin[:].rearrange("(p f) -> p f", p=128), hnewTh[:]
    )
    agout = bdram.tile([NC, 128 * 2 * JL * B], F16, addr_space="Shared",
                       tag="agout")
    nc.gpsimd.collective_compute(
        "AllGather", mybir.AluOpType.bypass,
        ins=[agin[:].opt()], outs=[agout[:].opt()],
        replica_groups=[list(range(NC))],
    )
    agr = agout[:].rearrange(
        "k (dp dt jl b) -> dt dp k jl b", dp=128, dt=2, jl=JL, b=B
    )
    for dt in range(2):
        nc.gpsimd.dma_start(
            hTh[dt][:].rearrange("dp (k jl) b -> dp k jl b", k=NC), agr[dt]
        )

    if DBG and t == 0:
        dbg = io["dbg"]
        for dt in range(2):
            nc.sync.dma_start(
                dbg[:, 128 * dt : 128 * (dt + 1)].rearrange(
                    "p (j b) -> p j b", j=JL
                ),
                hTown[dt][:],
            )
        nc.sync.dma_start(
            dbg[:, 256:512].rearrange("p (d j b) -> p d j b", d=2, j=JL),
            hrelu[:],
        )
        nc.sync.dma_start(
            dbg[:, 512:768].rearrange("p (d j b) -> p d j b", d=2, j=JL),
            hsum[:],
        )
        nc.sync.dma_start(
            dbg[0:1, 1024:1280].rearrange("p (m jb) -> p m jb", m=2), mrs[:]
        )
        nc.gpsimd.dma_start(
            dbg[:, 2048:2560].rearrange("p (g il jq b) -> p g il jq b",
                                        g=4, il=4, jq=4),
            inter[:, 0:4],
        )
        nc.gpsimd.dma_start(
            dbg[:, 2560:3072].rearrange("p (g jl jq b) -> p g jl jq b",
                                        g=4, jl=4, jq=4),
            inter2[:, 0:4],
        )
        nc.gpsimd.dma_start(
            dbg[:, 3072:4096].rearrange("p (s b) -> p s b", s=S),
            hTh[0][:],
        )


def _phase_c(nc, tc, io, ident, pid_v, hTh, qoT16):
    with tc.tile_pool(name="pc_sb", bufs=1) as pc, \
         tc.tile_pool(name="pc_ps", bufs=3, space="PSUM") as cps, \
         tc.tile_pool(name="pc_lg", bufs=4, space="PSUM") as lgps, \
         tc.tile_pool(name="pc_wo", bufs=3) as wop, \
         tc.tile_pool(name="pc_lgsb", bufs=2) as lgsb:

        wkf_sb = pc.tile([128, 2, D], F16)
        wvf_sb = pc.tile([128, 2, D], F16)
        for ct in range(2):
            nc.sync.dma_start(
                wkf_sb[:, ct], io["wkf16"][128 * ct : 128 * (ct + 1), :]
            )
            nc.sync.dma_start(
                wvf_sb[:, ct], io["wvf16"][128 * ct : 128 * (ct + 1), :]
            )

        # own-batch h^T slice (dynamic b=pid) -> static tiles
        hb = [pc.tile([128, S], F16, name=f"hb{dt}") for dt in range(2)]
        for dt in range(2):
            nc.vector.tensor_copy(
                hb[dt][:].rearrange("p (s o) -> p s o", o=1),
                hTh[dt][:, :, bass.ds(pid_v, 1)],
            )

        # K_f^T [d'128 x2, s128] ; V_f natural [s, d'] (fp16)
        kfT = [pc.tile([128, S], F16, name=f"kfT{pt}") for pt in range(2)]
        for pt in range(2):
            ps = cps.tile([128, L], F32, tag="c")
            for ct in range(2):
                nc.tensor.matmul(
                    ps[:, 0:S], wkf_sb[:, ct, 128 * pt : 128 * (pt + 1)],
                    hb[ct][:],
                    start=(ct == 0), stop=(ct == 1),
                )
            nc.vector.tensor_copy(kfT[pt][:], ps[:, 0:S])
        vf = pc.tile([S, D], F16)
        psv = cps.tile([128, L], F32, tag="c")
        for ct in range(2):
            nc.tensor.matmul(
                psv[0:S, 0:D], hb[ct][:], wvf_sb[:, ct],
                start=(ct == 0), stop=(ct == 1),
            )
        nc.vector.tensor_copy(vf[:], psv[0:S, 0:D])

        # expand attention -> A2^T [s, t512] (fp16)
        a2T = pc.tile([S, L], F16)
        for tt in range(4):
            sc = cps.tile([128, L], F32, tag="c")
            for pt in range(2):
                nc.tensor.matmul(
                    sc[:, 0:S], qoT16[pt][:, 128 * tt : 128 * (tt + 1)], kfT[pt][:],
                    start=(pt == 0), stop=(pt == 1),
                )
            rowmax = pc.tile([128, 1], F32, tag="rmax2")
            nc.vector.tensor_reduce(
                rowmax[:], sc[:, 0:S], axis=mybir.AxisListType.X,
                op=mybir.AluOpType.max,
            )
            nb = pc.tile([128, 1], F32, tag="nb2")
            nc.vector.tensor_scalar_mul(nb[:], rowmax[:], -SCALE)
            a2 = pc.tile([128, S], F32, tag="a2")
            sumexp = pc.tile([128, 1], F32, tag="sexp2")
            nc.scalar.activation(
                a2[:], sc[:, 0:S], mybir.ActivationFunctionType.Exp,
                bias=nb[:], scale=SCALE, accum_out=sumexp[:],
            )
            rs = pc.tile([128, 1], F32, tag="rs2")
            nc.vector.reciprocal(rs[:], sumexp[:])
            nc.vector.tensor_scalar_mul(a2[:], a2[:], rs[:])
            ptr = cps.tile([128, L], F32, tag="c")
            nc.tensor.transpose(ptr[:, 0:S], a2[:], ident[:])
            nc.vector.tensor_copy(a2T[:, 128 * tt : 128 * (tt + 1)], ptr[:, 0:S])

        # Y^T [d128 x2, t512] (fp16)
        yT = [pc.tile([128, L], F16, name=f"yT{dt}") for dt in range(2)]
        for dt in range(2):
            ps = cps.tile([128, L], F32, tag="c")
            nc.tensor.matmul(
                ps[:], vf[:, 128 * dt : 128 * (dt + 1)], a2T[:],
                start=True, stop=True,
            )
            nc.vector.tensor_copy(yT[dt][:], ps[:])

        # logits: out[t128, v500] per (vt, tt); batched store per vt
        for vt in range(NVT):
            wo_sb = wop.tile([128, 2, VT], F16, tag="wo")
            nc.scalar.dma_start(wo_sb[:], io["wout16"][vt])
            lg_sb = lgsb.tile([128, 4, VT], F32, tag="lg_sb")
            for tt in range(4):
                lg = lgps.tile([128, VT], F32, tag="lg")
                for dt in range(2):
                    nc.tensor.matmul(
                        lg[:],
                        yT[dt][:, 128 * tt : 128 * (tt + 1)],
                        wo_sb[:, dt, :],
                        start=(dt == 0), stop=(dt == 1),
                    )
                nc.any.tensor_copy(lg_sb[:, tt, :], lg[:])
            nc.sync.dma_start(
                io["lg_out"].rearrange("(tt p) v -> p tt v", tt=4)[
                    :, :, VT * vt : VT * (vt + 1)
                ],
                lg_sb[:],
            )


# ---------------------------------------------------------------------------
# Host side
# ---------------------------------------------------------------------------

_NC_CACHE = {}


def _get_nc():
    key = N_STEPS
    if key not in _NC_CACHE:
        _NC_CACHE[key] = build()
    return _NC_CACHE[key]


def _prep_in_maps(inputs):
    f32 = lambda a: np.ascontiguousarray(np.asarray(a), dtype=np.float32)
    input_ids = np.asarray(inputs["input_ids"])
    attention_mask = np.asarray(inputs["attention_mask"])
    H = f32(inputs["H"])
    W_source = np.asarray(inputs["W_source"], dtype=np.float32)
    W_target = np.asarray(inputs["W_target"], dtype=np.float32)

    # lngbT[p, t, 0:2] = gamma[t, 128*dt+p]; [p, t, 2:4] = beta
    lngbT = np.zeros((128, STEPS, 4), dtype=np.float32)
    lnsc = np.asarray(inputs["ln_scale"], dtype=np.float32).reshape(STEPS, 2, 128)
    lnbi = np.asarray(inputs["ln_bias"], dtype=np.float32).reshape(STEPS, 2, 128)
    lngbT[:, :, 0:2] = lnsc.transpose(2, 0, 1)
    lngbT[:, :, 2:4] = lnbi.transpose(2, 0, 1)

    rep = {
        "wqT": f32(np.asarray(inputs["Wq_in"]).T),
        "wkslT": f32(np.asarray(inputs["Wk_slots"]).T),
        "wvT": f32(np.asarray(inputs["Wv_in"]).T),
        "wqoT": f32(np.asarray(inputs["Wq_out"]).T),
        "wkf16": np.ascontiguousarray(
            np.asarray(inputs["Wk_fin"]).T.astype(np.float16)
        ),
        "wvf16": np.ascontiguousarray(
            np.asarray(inputs["Wv_fin"]).T.astype(np.float16)
        ),
        "hT_in": f32(H.T),
        "hn_in": H,
        "lngbT": lngbT,
        # wout16[vt, dp, dt, v] = Wout[500vt+v, 128dt+dp]
        "wout16": np.ascontiguousarray(
            f32(inputs["W_out_proj"]).reshape(NVT, VT, 2, 128)
            .transpose(0, 3, 2, 1).astype(np.float16)
        ),
    }

    in_maps = []
    for c in range(NC):
        m = dict(rep)
        X = (np.asarray(inputs["token_emb"], dtype=np.float32)[input_ids[c]]
             + np.asarray(inputs["pos_emb"], dtype=np.float32))
        m["xT_in"] = np.ascontiguousarray(X.T)
        m["maskw"] = np.ascontiguousarray(
            attention_mask[c].astype(np.float32).reshape(4, 128).T
        )
        # wsrc16[g, dp, il, jq, dt, (jloc r)] = Ws[4g+il, 16c+4jq+jloc, 128dt+dp, r]
        ws = W_source[:, JL * c : JL * (c + 1)]      # [128 i, 16 j, 256 d, 32 r]
        ws = ws.reshape(G, 4, 4, 4, 2, 128, R)       # [g, il, jq, jloc, dt, dp, r]
        ws = ws.transpose(0, 5, 1, 2, 4, 3, 6)       # [g, dp, il, jq, dt, jloc, r]
        m["wsrc16"] = np.ascontiguousarray(
            ws.reshape(G, 128, 4, 4, 2, 128).astype(np.float16)
        )
        # wtgt16[g, (il r), j, dh, dc] = Wt[4g+il, 16c+j, r, 128dh+dc], diag zeroed
        wt = W_target[:, JL * c : JL * (c + 1)].copy()   # [128 i, 16 j, 32 r, 256 d]
        for jl in range(JL):
            wt[JL * c + jl, jl] = 0.0
        wt = wt.reshape(G, 4, JL, R, 2, 128)         # [g, il, j, r, dh, dc]
        wt = wt.transpose(0, 1, 3, 2, 4, 5)          # [g, il, r, j, dh, dc]
        m["wtgt16"] = np.ascontiguousarray(
            wt.reshape(G, 128, JL, 2, 128).astype(np.float16)
        )
        in_maps.append(m)
    return in_maps


def run(inputs, trace=False):
    nc = _get_nc()
    in_maps = _prep_in_maps(inputs)
    res = bass_utils.run_bass_kernel_spmd(
        nc, in_maps, core_ids=list(range(NC)), trace=trace
    )
    out = np.stack([res.results[c]["lg_out"] for c in range(NC)], axis=0)
    return out, res


def kernel(**inputs):
    out, _ = run(inputs, trace=False)
    return out


# revision 15
# speedup vs baseline: 2.9352x; 1.1670x over previous
"""Trainium2 Bass kernel for nn_ConnectionTransformer (8 NeuronCores, SPMD).

Strategy (v2)
-------------
- Phase A (embed + compress attention): batch-parallel, core c handles batch c.
  fp32 math; produces the replicated transposed slot state.
- Phase B (6 bilinear message-passing steps): target-slot sharding - core c owns
  16 target slots j. Per-pair weights are cast to fp16 on the host (rel err
  ~5e-4 on the final logits, far under the 2e-2 gate) which halves the HBM
  stream (67 MB/core/step) AND makes every matmul a 1-cycle/row fp16 op with
  FWL weight loads. Both einsums are arranged so the streamed weight tile is
  the 128x128 stationary operand and the B=8 batch is the moving operand
  (8-col streams), so the PE cost is LDWEIGHTS-bound at ~55 us/step/einsum -
  under the ~190 us/step DMA floor. The step is therefore memory-bound.
- einsum1 emits inter[(jloc,r), g, il, jq, b]; an SBUF->SBUF DMA regroups it
  to [(il,r), g, jloc, jq, b] so einsum2 can contract (il,r)=128 per
  (j, g, dh) with full-height stationary tiles.
- relu/residual/LayerNorm run entirely in the transposed (d-on-partitions)
  layout: partition-dim sums via ones-matmuls, per-(j,b) mean/rstd broadcast
  back with a 1-row ones matmul, affine via tensor_scalar with per-partition
  gamma/beta. Output is already the hT layout the next step needs - no
  per-slot PE transposes.
- Each step AllGathers the 16 updated slots in fp16 (64 KB/core); the
  residual path keeps the core's own slots in fp32 locally.
- Phase C (expand attention + vocab projection): batch-parallel, fp16
  operands for the big matmuls, fp32 softmax/logits. Logits stores are
  batched per vocab tile ([128,4,500] = 1 MB per DMA).

Queueing: weight streams ride the SP (wsrc) and Activation (wtgt) HWDGE
queues so the two streams self-sequence; collective bounces ride the Pool
SWDGE queue so they never block next-step weight prefetch.
"""
import os
import sys

sys.path.insert(0, "/opt/trn_rl_repo")

import numpy as np
from concourse import bass, bacc, tile, bass_utils, mybir
from concourse import masks

B, L, D, S, R, STEPS, V = 8, 512, 256, 128, 32, 6, 32000
NC = 8
JL = S // NC          # 16 local target slots per core
G = S // 4            # 32 chunks of 4 source slots
VT = 500              # vocab tile width
NVT = V // VT         # 64 vocab tiles
SCALE = 1.0 / np.sqrt(D)
LN_EPS = 1e-5

F32 = mybir.dt.float32
F16 = mybir.dt.float16

N_STEPS = int(os.environ.get("N_STEPS", str(STEPS)))
DBG = bool(int(os.environ.get("DBG", "0")))


# ---------------------------------------------------------------------------
# Device program
# ---------------------------------------------------------------------------

def build():
    nc = bacc.Bacc("TRN2", target_bir_lowering=False, debug=False, num_devices=NC)

    io = {}

    def inp(name, shape, dtype=F32):
        io[name] = nc.dram_tensor(name, shape, dtype, kind="ExternalInput").ap()

    inp("xT_in", [D, L])
    inp("maskw", [128, 4])
    for w in ("wqT", "wkslT", "wvT", "wqoT"):
        inp(w, [D, D])
    inp("wkf16", [D, D], F16)
    inp("wvf16", [D, D], F16)
    inp("hT_in", [D, S])
    inp("hn_in", [S, D])
    inp("lngbT", [128, STEPS, 4])
    inp("wsrc16", [G, 128, 4, 4, 2, 128], F16)
    inp("wtgt16", [G, 128, JL, 2, 128], F16)
    inp("wout16", [NVT, 128, 2, VT], F16)
    io["lg_out"] = nc.dram_tensor(
        "lg_out", [L, V], F16, kind="ExternalOutput"
    ).ap()
    if DBG:
        io["dbg"] = nc.dram_tensor(
            "dbg", [128, 4096], F32, kind="ExternalOutput"
        ).ap()

    with tile.TileContext(nc) as tc:
        _body(nc, tc, io)
    nc.compile()
    return nc


def _body(nc, tc, io):
    with tc.tile_pool(name="const", bufs=1) as const, \
         tc.tile_pool(name="state", bufs=1) as state:

        ident = const.tile([128, 128], F32)
        masks.make_identity(nc, ident[:])
        ones = const.tile([128, 1], F32)
        nc.vector.memset(ones[:], 1.0)
        ones_row = const.tile([1, 128], F32)
        nc.vector.memset(ones_row[:], 1.0)
        eps_sb = const.tile([128, 1], F32)
        nc.vector.memset(eps_sb[:], LN_EPS)

        # persistent state
        hTh = [state.tile([128, S, B], F16, name=f"hTh{dt}") for dt in range(2)]
        hTown = [state.tile([128, JL, B], F32, name=f"hTo{dt}") for dt in range(2)]
        qoT16 = [state.tile([128, L], F16, name=f"qoT{pt}") for pt in range(2)]
        lngbT_sb = state.tile([128, STEPS, 4], F32)
        nc.sync.dma_start(lngbT_sb[:], io["lngbT"])

        pid_v = nc.vector.partition_id()

        _phase_a(nc, tc, io, ident, ones, pid_v, hTh, hTown, qoT16)
        with tc.tile_pool(name="ws", bufs=6) as wsp, \
             tc.tile_pool(name="wt", bufs=4) as wtp, \
             tc.tile_pool(name="istep", bufs=2) as isp, \
             tc.tile_pool(name="lnsb", bufs=2) as lnsb, \
             tc.tile_pool(name="p1", bufs=2, space="PSUM") as p1p, \
             tc.tile_pool(name="infl", bufs=1, space="PSUM") as inflp, \
             tc.tile_pool(name="lnps", bufs=1, space="PSUM") as lnps, \
             tc.tile_pool(name="bdram", bufs=2, space="DRAM") as bdram:
            for t in range(N_STEPS):
                _step(nc, tc, t, io, hTh, hTown, lngbT_sb, ones, ones_row,
                      eps_sb, wsp, wtp, isp, lnsb, p1p, inflp, lnps, bdram)
        _phase_c(nc, tc, io, ident, pid_v, hTh, qoT16)


def _phase_a(nc, tc, io, ident, ones, pid_v, hTh, hTown, qoT16):
    with tc.tile_pool(name="pa_sb", bufs=1) as pa, \
         tc.tile_pool(name="pa_ps", bufs=3, space="PSUM") as pps, \
         tc.tile_pool(name="pa_tp", bufs=2, space="PSUM") as tps, \
         tc.tile_pool(name="pa_acc", bufs=1, space="PSUM") as aps, \
         tc.tile_pool(name="dram_a", bufs=1, space="DRAM") as dra:

        mask_sb = pa.tile([128, 4], F32)
        nc.sync.dma_start(mask_sb[:], io["maskw"])

        # X^T tiles [d128, t512] (host-gathered embeddings, transposed)
        xT = [pa.tile([128, L], F32, name=f"xT{ct}") for ct in range(2)]
        for ct in range(2):
            nc.sync.dma_start(xT[ct][:], io["xT_in"][128 * ct : 128 * (ct + 1), :])

        def load_w(name):
            ts = [pa.tile([128, D], F32, name=f"{name}_{ct}") for ct in range(2)]
            for ct in range(2):
                nc.sync.dma_start(ts[ct][:], io[name][128 * ct : 128 * (ct + 1), :])
            return ts

        wq_sb = load_w("wqT")
        wv_sb = load_w("wvT")
        wksl_sb = load_w("wkslT")
        wqo_sb = load_w("wqoT")
        hTt = [pa.tile([128, S], F32, name=f"hTt{ct}") for ct in range(2)]
        for ct in range(2):
            nc.sync.dma_start(hTt[ct][:], io["hT_in"][128 * ct : 128 * (ct + 1), :])
        hn_sb = pa.tile([S, D], F32)
        nc.sync.dma_start(hn_sb[:], io["hn_in"])

        # Q_in^T and Q_out^T : [d'128 x 2, t512]
        qT = [pa.tile([128, L], F32, name=f"qT{pt}") for pt in range(2)]
        qoT = [pa.tile([128, L], F32, name=f"qoTf{pt}") for pt in range(2)]
        for pt in range(2):
            for dst, wsb in ((qT, wq_sb), (qoT, wqo_sb)):
                ps = pps.tile([128, L], F32, tag="ps")
                for ct in range(2):
                    nc.tensor.matmul(
                        ps[:], wsb[ct][:, 128 * pt : 128 * (pt + 1)], xT[ct][:],
                        start=(ct == 0), stop=(ct == 1),
                    )
                nc.vector.tensor_copy(dst[pt][:], ps[:])
            nc.vector.tensor_copy(qoT16[pt][:], qoT[pt][:])

        # V_in natural [t128 x 4, d256]
        vn = pa.tile([128, 4, D], F32)
        for tt in range(4):
            ps = pps.tile([128, L], F32, tag="ps")
            for ct in range(2):
                nc.tensor.matmul(
                    ps[:, 0:D], xT[ct][:, 128 * tt : 128 * (tt + 1)], wv_sb[ct][:],
                    start=(ct == 0), stop=(ct == 1),
                )
            nc.vector.tensor_copy(vn[:, tt, :], ps[:, 0:D])

        # K_slots^T [d'128 x 2, s128]
        kslT = [pa.tile([128, S], F32, name=f"kslT{pt}") for pt in range(2)]
        for pt in range(2):
            ps = pps.tile([128, L], F32, tag="ps")
            for ct in range(2):
                nc.tensor.matmul(
                    ps[:, 0:S], wksl_sb[ct][:, 128 * pt : 128 * (pt + 1)], hTt[ct][:],
                    start=(ct == 0), stop=(ct == 1),
                )
            nc.vector.tensor_copy(kslT[pt][:], ps[:, 0:S])

        # attention scores + masked softmax
        a_sb = pa.tile([128, 4, S], F32)
        for tt in range(4):
            sc = pps.tile([128, L], F32, tag="ps")
            for pt in range(2):
                nc.tensor.matmul(
                    sc[:, 0:S], qT[pt][:, 128 * tt : 128 * (tt + 1)], kslT[pt][:],
                    start=(pt == 0), stop=(pt == 1),
                )
            rowmax = pa.tile([128, 1], F32, tag="rmax")
            nc.vector.tensor_reduce(
                rowmax[:], sc[:, 0:S], axis=mybir.AxisListType.X,
                op=mybir.AluOpType.max,
            )
            nb = pa.tile([128, 1], F32, tag="nb")
            nc.vector.tensor_scalar_mul(nb[:], rowmax[:], -SCALE)
            sumexp = pa.tile([128, 1], F32, tag="sexp")
            nc.scalar.activation(
                a_sb[:, tt, :], sc[:, 0:S], mybir.ActivationFunctionType.Exp,
                bias=nb[:], scale=SCALE, accum_out=sumexp[:],
            )
            rs = pa.tile([128, 1], F32, tag="rs")
            nc.vector.reciprocal(rs[:], sumexp[:])
            rm = pa.tile([128, 1], F32, tag="rmk")
            nc.vector.tensor_tensor(
                rm[:], rs[:], mask_sb[:, tt : tt + 1], op=mybir.AluOpType.mult
            )
            nc.vector.tensor_scalar_mul(a_sb[:, tt, :], a_sb[:, tt, :], rm[:])

        # column sums and IR = A^T @ V
        cs = aps.tile([128, 1], F32, tag="cs")
        for tt in range(4):
            nc.tensor.matmul(
                cs[:], a_sb[:, tt, :], ones[:, 0:1], start=(tt == 0), stop=(tt == 3)
            )
        ir = aps.tile([128, D], F32, tag="ir")
        for tt in range(4):
            nc.tensor.matmul(
                ir[:], a_sb[:, tt, :], vn[:, tt, :], start=(tt == 0), stop=(tt == 3)
            )
        cssb = pa.tile([128, 1], F32)
        nc.vector.tensor_scalar_add(cssb[:], cs[:], 1e-8)
        rcs = pa.tile([128, 1], F32)
        nc.vector.reciprocal(rcs[:], cssb[:])
        h0 = pa.tile([S, D], F32)
        nc.vector.scalar_tensor_tensor(
            h0[:], ir[:], rcs[:], hn_sb[:],
            op0=mybir.AluOpType.mult, op1=mybir.AluOpType.add,
        )

        # h0 -> transposed bounce, init AllGather (fp32, 2 segments)
        agin0 = dra.tile([2 * 16384], F32)
        for dt in range(2):
            p3 = tps.tile([128, 128], F32, tag="tp")
            nc.tensor.transpose(p3[:], h0[:, 128 * dt : 128 * (dt + 1)], ident[:])
            h0T = pa.tile([128, 128], F32, tag="h0T")
            nc.vector.tensor_copy(h0T[:], p3[:])
            nc.sync.dma_start(
                agin0[dt * 16384 : (dt + 1) * 16384].rearrange(
                    "(p f) -> p f", p=128
                ),
                h0T[:],
            )
        agout0 = dra.tile([NC, 2 * 16384], F32, addr_space="Shared")
        nc.gpsimd.collective_compute(
            "AllGather", mybir.AluOpType.bypass,
            ins=[agin0[:].opt()], outs=[agout0[:].opt()],
            replica_groups=[list(range(NC))],
        )
        # readback: batch-major bounce (contiguous descriptors), then strided
        # casts into the [dp, s, b] layouts
        ag0r = agout0[:].rearrange(
            "b (seg dp s) -> seg dp b s", seg=2, dp=128, s=128
        )
        hA = [pa.tile([128, B, S], F32, name=f"hA{dt}") for dt in range(2)]
        tmp = pa.tile([128, B, JL], F32)
        for dt in range(2):
            nc.sync.dma_start(hA[dt][:], ag0r[dt])
            nc.vector.tensor_copy(
                hTh[dt][:], hA[dt][:].rearrange("p b s -> p s b")
            )
            nc.vector.tensor_copy(
                tmp[:], hA[dt][:, :, bass.ds(pid_v * JL, JL)]
            )
            nc.vector.tensor_copy(
                hTown[dt][:], tmp[:].rearrange("p b j -> p j b")
            )


def _step(nc, tc, t, io, hTh, hTown, lngbT_sb, ones, ones_row, eps_sb,
          wsp, wtp, isp, lnsb, p1p, inflp, lnps, bdram):
    """One message-passing step (fp16 weights, transposed-layout LN)."""
    # whole-step inter buffers: [p, g, il, jq, b] and regrouped [p, g, jloc, jq, b]
    inter = isp.tile([128, G, 4, 4, B], F16, tag="inter")
    inter2 = isp.tile([128, G, 4, 4, B], F16, tag="inter2")
    inflT = inflp.tile([128, 2, JL, B], F32, tag="inflT")

    # ---- einsum1: inter[(jloc,r), g, il, jq, b] ----
    for g in range(G):
        ws = wsp.tile([128, 4, 4, 2, 128], F16, tag="ws")
        nc.sync.dma_start(ws[:], io["wsrc16"][g])
        p1 = p1p.tile([128, 4, 4, B], F32, tag="p1")
        for il in range(4):
            i = 4 * g + il
            for jq in range(4):
                for dt in range(2):
                    nc.tensor.matmul(
                        p1[:, il, jq, :],
                        ws[:, il, jq, dt, :],
                        hTh[dt][:, i, :],
                        start=(dt == 0), stop=(dt == 1),
                    )
        nc.vector.tensor_copy(inter[:, g, :, :, :], p1[:])

    # ---- regroup: (jloc,r) bands -> (il,r) bands (SBUF->SBUF DMA) ----
    for il in range(4):
        for jloc in range(4):
            nc.sync.dma_start(
                inter2[32 * il : 32 * (il + 1), :, jloc, :, :],
                inter[32 * jloc : 32 * (jloc + 1), :, il, :, :],
            )

    # ---- einsum2: inflT[dc, dh, j, b] += inter2 @ W_target ----
    # PSUM start_tensor_calc marks the whole 2KB zero region (= the bank
    # holding all 32 (j, dh) accumulators) pending-zero, so exactly ONE
    # start on the first matmul and ONE stop on the last - a per-group
    # start would wipe the other groups' partial sums.
    for g in range(G):
        wt = wtp.tile([128, JL, 2, 128], F16, tag="wt")
        nc.scalar.dma_start(wt[:], io["wtgt16"][g])
        for j in range(JL):
            jq, jloc = j // 4, j % 4
            for dh in range(2):
                nc.tensor.matmul(
                    inflT[:, dh, j, :],
                    wt[:, j, dh, :],
                    inter2[:, g, jloc, jq, :],
                    start=(g == 0 and j == 0 and dh == 0),
                    stop=(g == G - 1 and j == JL - 1 and dh == 1),
                    skip_group_check=True,
                )

    # ---- relu + residual + LayerNorm, all in transposed layout ----
    hrelu = lnsb.tile([128, 2, JL, B], F32, tag="hrelu")
    nc.scalar.activation(hrelu[:], inflT[:], mybir.ActivationFunctionType.Relu)
    hsum = lnsb.tile([128, 2, JL, B], F32, tag="hsum")
    for dt in range(2):
        nc.vector.tensor_tensor(
            hsum[:, dt], hrelu[:, dt], hTown[dt][:], op=mybir.AluOpType.add
        )
    sq = lnsb.tile([128, 2, JL, B], F32, tag="sq")
    nc.vector.tensor_tensor(sq[:], hsum[:], hsum[:], op=mybir.AluOpType.mult)
    sums = lnps.tile([1, 2, 2, JL, B], F32, tag="sums")  # [1, (s/sq), dt, j, b]
    nc.tensor.matmul(
        sums[0:1, 0], ones[:, 0:1], hsum[:], start=True, stop=True,
    )
    nc.tensor.matmul(
        sums[0:1, 1], ones[:, 0:1], sq[:], start=True, stop=True,
    )
    # mean/rstd per (j, b): combine dt halves on 1 partition
    sums_sb = lnsb.tile([1, 2, 2, JL, B], F32, tag="sums_sb")
    nc.vector.tensor_copy(sums_sb[:], sums[:])
    mrs = lnsb.tile([1, 2, JL * B], F32, tag="mrs")  # [1, (mean, rstd), jb]
    mean = mrs[0:1, 0]
    nc.vector.tensor_tensor(
        mean, sums_sb[0:1, 0, 0].rearrange("p j b -> p (j b)"),
        sums_sb[0:1, 0, 1].rearrange("p j b -> p (j b)"), op=mybir.AluOpType.add,
    )
    nc.vector.tensor_scalar_mul(mean, mean, 1.0 / D)
    ssq = lnsb.tile([1, JL * B], F32, tag="ssq")
    nc.vector.tensor_tensor(
        ssq[:], sums_sb[0:1, 1, 0].rearrange("p j b -> p (j b)"),
        sums_sb[0:1, 1, 1].rearrange("p j b -> p (j b)"), op=mybir.AluOpType.add,
    )
    nc.vector.tensor_scalar_mul(ssq[:], ssq[:], 1.0 / D)
    msq = lnsb.tile([1, JL * B], F32, tag="msq")
    nc.vector.tensor_tensor(msq[:], mean, mean, op=mybir.AluOpType.mult)
    var = lnsb.tile([1, JL * B], F32, tag="var")
    nc.vector.tensor_tensor(var[:], ssq[:], msq[:], op=mybir.AluOpType.subtract)
    std = lnsb.tile([1, JL * B], F32, tag="std")
    nc.scalar.activation(
        std[:], var[:], mybir.ActivationFunctionType.Sqrt, bias=eps_sb[0:1, :]
    )
    rstd = mrs[0:1, 1]
    nc.vector.reciprocal(rstd, std[:])
    # broadcast mean/rstd across partitions via 1-row ones matmul
    bc = lnps.tile([128, 2, JL * B], F32, tag="bc")
    nc.tensor.matmul(
        bc[:], ones_row[:], mrs[0:1].rearrange("p m jb -> p (m jb)"),
        start=True, stop=True,
    )
    # normalize + affine; write fp32 own-state and fp16 gather input
    hnewTh = lnsb.tile([128, 2, JL, B], F16, tag="hnewTh")
    cen = lnsb.tile([128, JL, B], F32, tag="cen")
    for dt in range(2):
        nc.vector.tensor_tensor(
            cen[:], hsum[:, dt],
            bc[:, 0].rearrange("p (j b) -> p j b", j=JL),
            op=mybir.AluOpType.subtract,
        )
        nc.vector.tensor_tensor(
            cen[:], cen[:],
            bc[:, 1].rearrange("p (j b) -> p j b", j=JL),
            op=mybir.AluOpType.mult,
        )
        nc.vector.tensor_scalar(
            hTown[dt][:], cen[:],
            lngbT_sb[:, t, dt : dt + 1],
            lngbT_sb[:, t, 2 + dt : 3 + dt],
            op0=mybir.AluOpType.mult, op1=mybir.AluOpType.add,
        )
        nc.vector.tensor_copy(hnewTh[:, dt], hTown[dt][:])

    # ---- AllGather the 16 updated slots (fp16) ----
    agin = bdram.tile([128 * 2 * JL * B], F16, tag="agin")
    nc.gpsimd.dma_start(
        agin[:].rearrange("(p f) -> p f", p=128), hnewTh[:]
    )
    agout = bdram.tile([NC, 128 * 2 * JL * B], F16, addr_space="Shared",
                       tag="agout")
    nc.gpsimd.collective_compute(
        "AllGather", mybir.AluOpType.bypass,
        ins=[agin[:].opt()], outs=[agout[:].opt()],
        replica_groups=[list(range(NC))],
    )
    agr = agout[:].rearrange(
        "k (dp dt jl b) -> dt dp k jl b", dp=128, dt=2, jl=JL, b=B
    )
    for dt in range(2):
        nc.gpsimd.dma_start(
            hTh[dt][:].rearrange("dp (k jl) b -> dp k jl b", k=NC), agr[dt]
        )

    if DBG and t == 0:
        dbg = io["dbg"]
        for dt in range(2):
            nc.sync.dma_start(
                dbg[:, 128 * dt : 128 * (dt + 1)].rearrange(
                    "p (j b) -> p j b", j=JL
                ),
                hTown[dt][:],
            )
        nc.sync.dma_start(
            dbg[:, 256:512].rearrange("p (d j b) -> p d j b", d=2, j=JL),
            hrelu[:],
        )
        nc.sync.dma_start(
            dbg[:, 512:768].rearrange("p (d j b) -> p d j b", d=2, j=JL),
            hsum[:],
        )
        nc.sync.dma_start(
            dbg[0:1, 1024:1280].rearrange("p (m jb) -> p m jb", m=2), mrs[:]
        )
        nc.gpsimd.dma_start(
            dbg[:, 2048:2560].rearrange("p (g il jq b) -> p g il jq b",
                                        g=4, il=4, jq=4),
            inter[:, 0:4],
        )
        nc.gpsimd.dma_start(
            dbg[:, 2560:3072].rearrange("p (g jl jq b) -> p g jl jq b",
                                        g=4, jl=4, jq=4),
            inter2[:, 0:4],
        )
        nc.gpsimd.dma_start(
            dbg[:, 3072:4096].rearrange("p (s b) -> p s b", s=S),
            hTh[0][:],
        )


def _phase_c(nc, tc, io, ident, pid_v, hTh, qoT16):
    with tc.tile_pool(name="pc_sb", bufs=1) as pc, \
         tc.tile_pool(name="pc_ps", bufs=3, space="PSUM") as cps, \
         tc.tile_pool(name="pc_lg", bufs=4, space="PSUM") as lgps, \
         tc.tile_pool(name="pc_wo", bufs=3) as wop, \
         tc.tile_pool(name="pc_lgsb", bufs=3) as lgsb:

        wkf_sb = pc.tile([128, 2, D], F16)
        wvf_sb = pc.tile([128, 2, D], F16)
        for ct in range(2):
            nc.sync.dma_start(
                wkf_sb[:, ct], io["wkf16"][128 * ct : 128 * (ct + 1), :]
            )
            nc.sync.dma_start(
                wvf_sb[:, ct], io["wvf16"][128 * ct : 128 * (ct + 1), :]
            )

        # own-batch h^T slice (dynamic b=pid) -> static tiles
        hb = [pc.tile([128, S], F16, name=f"hb{dt}") for dt in range(2)]
        for dt in range(2):
            nc.vector.tensor_copy(
                hb[dt][:].rearrange("p (s o) -> p s o", o=1),
                hTh[dt][:, :, bass.ds(pid_v, 1)],
            )

        # K_f^T [d'128 x2, s128] ; V_f natural [s, d'] (fp16)
        kfT = [pc.tile([128, S], F16, name=f"kfT{pt}") for pt in range(2)]
        for pt in range(2):
            ps = cps.tile([128, L], F32, tag="c")
            for ct in range(2):
                nc.tensor.matmul(
                    ps[:, 0:S], wkf_sb[:, ct, 128 * pt : 128 * (pt + 1)],
                    hb[ct][:],
                    start=(ct == 0), stop=(ct == 1),
                )
            nc.vector.tensor_copy(kfT[pt][:], ps[:, 0:S])
        vf = pc.tile([S, D], F16)
        psv = cps.tile([128, L], F32, tag="c")
        for ct in range(2):
            nc.tensor.matmul(
                psv[0:S, 0:D], hb[ct][:], wvf_sb[:, ct],
                start=(ct == 0), stop=(ct == 1),
            )
        nc.vector.tensor_copy(vf[:], psv[0:S, 0:D])

        # expand attention -> A2^T [s, t512] (fp16)
        a2T = pc.tile([S, L], F16)
        for tt in range(4):
            sc = cps.tile([128, L], F32, tag="c")
            for pt in range(2):
                nc.tensor.matmul(
                    sc[:, 0:S], qoT16[pt][:, 128 * tt : 128 * (tt + 1)], kfT[pt][:],
                    start=(pt == 0), stop=(pt == 1),
                )
            rowmax = pc.tile([128, 1], F32, tag="rmax2")
            nc.vector.tensor_reduce(
                rowmax[:], sc[:, 0:S], axis=mybir.AxisListType.X,
                op=mybir.AluOpType.max,
            )
            nb = pc.tile([128, 1], F32, tag="nb2")
            nc.vector.tensor_scalar_mul(nb[:], rowmax[:], -SCALE)
            a2 = pc.tile([128, S], F32, tag="a2")
            sumexp = pc.tile([128, 1], F32, tag="sexp2")
            nc.scalar.activation(
                a2[:], sc[:, 0:S], mybir.ActivationFunctionType.Exp,
                bias=nb[:], scale=SCALE, accum_out=sumexp[:],
            )
            rs = pc.tile([128, 1], F32, tag="rs2")
            nc.vector.reciprocal(rs[:], sumexp[:])
            nc.vector.tensor_scalar_mul(a2[:], a2[:], rs[:])
            ptr = cps.tile([128, L], F32, tag="c")
            nc.tensor.transpose(ptr[:, 0:S], a2[:], ident[:])
            nc.vector.tensor_copy(a2T[:, 128 * tt : 128 * (tt + 1)], ptr[:, 0:S])

        # Y^T [d128 x2, t512] (fp16)
        yT = [pc.tile([128, L], F16, name=f"yT{dt}") for dt in range(2)]
        for dt in range(2):
            ps = cps.tile([128, L], F32, tag="c")
            nc.tensor.matmul(
                ps[:], vf[:, 128 * dt : 128 * (dt + 1)], a2T[:],
                start=True, stop=True,
            )
            nc.vector.tensor_copy(yT[dt][:], ps[:])

        # logits: out[t128, v500] per (vt, tt); fp16 store batched per vt
        for vt in range(NVT):
            wo_sb = wop.tile([128, 2, VT], F16, tag="wo")
            nc.scalar.dma_start(wo_sb[:], io["wout16"][vt])
            lg_sb = lgsb.tile([128, 4, VT], F16, tag="lg_sb")
            for tt in range(4):
                lg = lgps.tile([128, VT], F32, tag="lg")
                for dt in range(2):
                    nc.tensor.matmul(
                        lg[:],
                        yT[dt][:, 128 * tt : 128 * (tt + 1)],
                        wo_sb[:, dt, :],
                        start=(dt == 0), stop=(dt == 1),
                    )
                nc.any.tensor_copy(lg_sb[:, tt, :], lg[:])
            nc.sync.dma_start(
                io["lg_out"].rearrange("(tt p) v -> p tt v", tt=4)[
                    :, :, VT * vt : VT * (vt + 1)
                ],
                lg_sb[:],
            )


# ---------------------------------------------------------------------------
# Host side
# ---------------------------------------------------------------------------

_NC_CACHE = {}


def _get_nc():
    key = N_STEPS
    if key not in _NC_CACHE:
        _NC_CACHE[key] = build()
    return _NC_CACHE[key]


def _prep_in_maps(inputs):
    f32 = lambda a: np.ascontiguousarray(np.asarray(a), dtype=np.float32)
    input_ids = np.asarray(inputs["input_ids"])
    attention_mask = np.asarray(inputs["attention_mask"])
    H = f32(inputs["H"])
    W_source = np.asarray(inputs["W_source"], dtype=np.float32)
    W_target = np.asarray(inputs["W_target"], dtype=np.float32)

    # lngbT[p, t, 0:2] = gamma[t, 128*dt+p]; [p, t, 2:4] = beta
    lngbT = np.zeros((128, STEPS, 4), dtype=np.float32)
    lnsc = np.asarray(inputs["ln_scale"], dtype=np.float32).reshape(STEPS, 2, 128)
    lnbi = np.asarray(inputs["ln_bias"], dtype=np.float32).reshape(STEPS, 2, 128)
    lngbT[:, :, 0:2] = lnsc.transpose(2, 0, 1)
    lngbT[:, :, 2:4] = lnbi.transpose(2, 0, 1)

    rep = {
        "wqT": f32(np.asarray(inputs["Wq_in"]).T),
        "wkslT": f32(np.asarray(inputs["Wk_slots"]).T),
        "wvT": f32(np.asarray(inputs["Wv_in"]).T),
        "wqoT": f32(np.asarray(inputs["Wq_out"]).T),
        "wkf16": np.ascontiguousarray(
            np.asarray(inputs["Wk_fin"]).T.astype(np.float16)
        ),
        "wvf16": np.ascontiguousarray(
            np.asarray(inputs["Wv_fin"]).T.astype(np.float16)
        ),
        "hT_in": f32(H.T),
        "hn_in": H,
        "lngbT": lngbT,
        # wout16[vt, dp, dt, v] = Wout[500vt+v, 128dt+dp]
        "wout16": np.ascontiguousarray(
            f32(inputs["W_out_proj"]).reshape(NVT, VT, 2, 128)
            .transpose(0, 3, 2, 1).astype(np.float16)
        ),
    }

    in_maps = []
    for c in range(NC):
        m = dict(rep)
        X = (np.asarray(inputs["token_emb"], dtype=np.float32)[input_ids[c]]
             + np.asarray(inputs["pos_emb"], dtype=np.float32))
        m["xT_in"] = np.ascontiguousarray(X.T)
        m["maskw"] = np.ascontiguousarray(
            attention_mask[c].astype(np.float32).reshape(4, 128).T
        )
        # wsrc16[g, dp, il, jq, dt, (jloc r)] = Ws[4g+il, 16c+4jq+jloc, 128dt+dp, r]
        ws = W_source[:, JL * c : JL * (c + 1)]      # [128 i, 16 j, 256 d, 32 r]
        ws = ws.reshape(G, 4, 4, 4, 2, 128, R)       # [g, il, jq, jloc, dt, dp, r]
        ws = ws.transpose(0, 5, 1, 2, 4, 3, 6)       # [g, dp, il, jq, dt, jloc, r]
        m["wsrc16"] = np.ascontiguousarray(
            ws.reshape(G, 128, 4, 4, 2, 128).astype(np.float16)
        )
        # wtgt16[g, (il r), j, dh, dc] = Wt[4g+il, 16c+j, r, 128dh+dc], diag zeroed
        wt = W_target[:, JL * c : JL * (c + 1)].copy()   # [128 i, 16 j, 32 r, 256 d]
        for jl in range(JL):
            wt[JL * c + jl, jl] = 0.0
        wt = wt.reshape(G, 4, JL, R, 2, 128)         # [g, il, j, r, dh, dc]
        wt = wt.transpose(0, 1, 3, 2, 4, 5)          # [g, il, r, j, dh, dc]
        m["wtgt16"] = np.ascontiguousarray(
            wt.reshape(G, 128, JL, 2, 128).astype(np.float16)
        )
        in_maps.append(m)
    return in_maps


def run(inputs, trace=False):
    nc = _get_nc()
    in_maps = _prep_in_maps(inputs)
    res = bass_utils.run_bass_kernel_spmd(
        nc, in_maps, core_ids=list(range(NC)), trace=trace
    )
    out = np.stack(
        [res.results[c]["lg_out"].astype(np.float32) for c in range(NC)], axis=0
    )
    return out, res


def kernel(**inputs):
    out, _ = run(inputs, trace=False)
    return out


# revision 19
# speedup vs baseline: 3.0550x; 1.0408x over previous
"""Trainium2 Bass kernel for nn_ConnectionTransformer (8 NeuronCores, SPMD).

Strategy (v2)
-------------
- Phase A (embed + compress attention): batch-parallel, core c handles batch c.
  fp32 math; produces the replicated transposed slot state.
- Phase B (6 bilinear message-passing steps): target-slot sharding - core c owns
  16 target slots j. Per-pair weights are cast to fp16 on the host (rel err
  ~5e-4 on the final logits, far under the 2e-2 gate) which halves the HBM
  stream (67 MB/core/step) AND makes every matmul a 1-cycle/row fp16 op with
  FWL weight loads. Both einsums are arranged so the streamed weight tile is
  the 128x128 stationary operand and the B=8 batch is the moving operand
  (8-col streams), so the PE cost is LDWEIGHTS-bound at ~55 us/step/einsum -
  under the ~190 us/step DMA floor. The step is therefore memory-bound.
- einsum1 emits inter[(jloc,r), g, il, jq, b]; an SBUF->SBUF DMA regroups it
  to [(il,r), g, jloc, jq, b] so einsum2 can contract (il,r)=128 per
  (j, g, dh) with full-height stationary tiles.
- relu/residual/LayerNorm run entirely in the transposed (d-on-partitions)
  layout: partition-dim sums via ones-matmuls, per-(j,b) mean/rstd broadcast
  back with a 1-row ones matmul, affine via tensor_scalar with per-partition
  gamma/beta. Output is already the hT layout the next step needs - no
  per-slot PE transposes.
- Each step AllGathers the 16 updated slots in fp16 (64 KB/core); the
  residual path keeps the core's own slots in fp32 locally.
- Phase C (expand attention + vocab projection): batch-parallel, fp16
  operands for the big matmuls, fp32 softmax/logits. Logits stores are
  batched per vocab tile ([128,4,500] = 1 MB per DMA).

Queueing: weight streams ride the SP (wsrc) and Activation (wtgt) HWDGE
queues so the two streams self-sequence; collective bounces ride the Pool
SWDGE queue so they never block next-step weight prefetch.
"""
import os
import sys

sys.path.insert(0, "/opt/trn_rl_repo")

import numpy as np
from concourse import bass, bacc, tile, bass_utils, mybir
from concourse import masks

B, L, D, S, R, STEPS, V = 8, 512, 256, 128, 32, 6, 32000
NC = 8
JL = S // NC          # 16 local target slots per core
G = S // 4            # 32 chunks of 4 source slots
VT = 500              # vocab tile width
NVT = V // VT         # 64 vocab tiles
SCALE = 1.0 / np.sqrt(D)
LN_EPS = 1e-5

F32 = mybir.dt.float32
F16 = mybir.dt.float16

N_STEPS = int(os.environ.get("N_STEPS", str(STEPS)))
DBG = bool(int(os.environ.get("DBG", "0")))


# ---------------------------------------------------------------------------
# Device program
# ---------------------------------------------------------------------------

def build():
    nc = bacc.Bacc("TRN2", target_bir_lowering=False, debug=False, num_devices=NC)

    io = {}

    def inp(name, shape, dtype=F32):
        io[name] = nc.dram_tensor(name, shape, dtype, kind="ExternalInput").ap()

    inp("xT_in", [D, L])
    inp("maskw", [128, 4])
    for w in ("wqT", "wkslT", "wvT", "wqoT"):
        inp(w, [D, D])
    inp("wkf16", [D, D], F16)
    inp("wvf16", [D, D], F16)
    inp("hT_in", [D, S])
    inp("hn_in", [S, D])
    inp("lngbT", [128, STEPS, 4])
    inp("wsrc16", [G, 128, 4, 4, 2, 128], F16)
    inp("wtgt16", [G, 128, JL, 2, 128], F16)
    inp("wout16", [NVT, 128, 2, VT], F16)
    io["lg_out"] = nc.dram_tensor(
        "lg_out", [L, V], F16, kind="ExternalOutput"
    ).ap()
    if DBG:
        io["dbg"] = nc.dram_tensor(
            "dbg", [128, 4096], F32, kind="ExternalOutput"
        ).ap()

    with tile.TileContext(nc) as tc:
        _body(nc, tc, io)
    nc.compile()
    return nc


def _body(nc, tc, io):
    with tc.tile_pool(name="const", bufs=1) as const, \
         tc.tile_pool(name="state", bufs=1) as state:

        ident = const.tile([128, 128], F32)
        masks.make_identity(nc, ident[:])
        ones = const.tile([128, 1], F32)
        nc.vector.memset(ones[:], 1.0)
        ones_row = const.tile([1, 128], F32)
        nc.vector.memset(ones_row[:], 1.0)
        eps_sb = const.tile([128, 1], F32)
        nc.vector.memset(eps_sb[:], LN_EPS)

        # persistent state
        hTh = [state.tile([128, S, B], F16, name=f"hTh{dt}") for dt in range(2)]
        hTown = [state.tile([128, JL, B], F32, name=f"hTo{dt}") for dt in range(2)]
        qoT16 = [state.tile([128, L], F16, name=f"qoT{pt}") for pt in range(2)]
        lngbT_sb = state.tile([128, STEPS, 4], F32)
        nc.sync.dma_start(lngbT_sb[:], io["lngbT"])

        pid_v = nc.vector.partition_id()

        _phase_a(nc, tc, io, ident, ones, pid_v, hTh, hTown, qoT16)
        with tc.tile_pool(name="ws", bufs=6) as wsp, \
             tc.tile_pool(name="wt", bufs=4) as wtp, \
             tc.tile_pool(name="istep", bufs=2) as isp, \
             tc.tile_pool(name="lnsb", bufs=2) as lnsb, \
             tc.tile_pool(name="p1", bufs=2, space="PSUM") as p1p, \
             tc.tile_pool(name="infl", bufs=1, space="PSUM") as inflp, \
             tc.tile_pool(name="lnps", bufs=1, space="PSUM") as lnps, \
             tc.tile_pool(name="bdram", bufs=2, space="DRAM") as bdram:
            for t in range(N_STEPS):
                _step(nc, tc, t, io, hTh, hTown, lngbT_sb, ones, ones_row,
                      eps_sb, wsp, wtp, isp, lnsb, p1p, inflp, lnps, bdram)
        _phase_c(nc, tc, io, ident, pid_v, hTh, qoT16)


def _phase_a(nc, tc, io, ident, ones, pid_v, hTh, hTown, qoT16):
    with tc.tile_pool(name="pa_sb", bufs=1) as pa, \
         tc.tile_pool(name="pa_ps", bufs=3, space="PSUM") as pps, \
         tc.tile_pool(name="pa_tp", bufs=2, space="PSUM") as tps, \
         tc.tile_pool(name="pa_acc", bufs=1, space="PSUM") as aps, \
         tc.tile_pool(name="dram_a", bufs=1, space="DRAM") as dra:

        mask_sb = pa.tile([128, 4], F32)
        nc.sync.dma_start(mask_sb[:], io["maskw"])

        # X^T tiles [d128, t512] (host-gathered embeddings, transposed)
        xT = [pa.tile([128, L], F32, name=f"xT{ct}") for ct in range(2)]
        for ct in range(2):
            nc.sync.dma_start(xT[ct][:], io["xT_in"][128 * ct : 128 * (ct + 1), :])

        def load_w(name):
            ts = [pa.tile([128, D], F32, name=f"{name}_{ct}") for ct in range(2)]
            for ct in range(2):
                nc.sync.dma_start(ts[ct][:], io[name][128 * ct : 128 * (ct + 1), :])
            return ts

        wq_sb = load_w("wqT")
        wv_sb = load_w("wvT")
        wksl_sb = load_w("wkslT")
        wqo_sb = load_w("wqoT")
        hTt = [pa.tile([128, S], F32, name=f"hTt{ct}") for ct in range(2)]
        for ct in range(2):
            nc.sync.dma_start(hTt[ct][:], io["hT_in"][128 * ct : 128 * (ct + 1), :])
        hn_sb = pa.tile([S, D], F32)
        nc.sync.dma_start(hn_sb[:], io["hn_in"])

        # Q_in^T and Q_out^T : [d'128 x 2, t512]
        qT = [pa.tile([128, L], F32, name=f"qT{pt}") for pt in range(2)]
        qoT = [pa.tile([128, L], F32, name=f"qoTf{pt}") for pt in range(2)]
        for pt in range(2):
            for dst, wsb in ((qT, wq_sb), (qoT, wqo_sb)):
                ps = pps.tile([128, L], F32, tag="ps")
                for ct in range(2):
                    nc.tensor.matmul(
                        ps[:], wsb[ct][:, 128 * pt : 128 * (pt + 1)], xT[ct][:],
                        start=(ct == 0), stop=(ct == 1),
                    )
                nc.vector.tensor_copy(dst[pt][:], ps[:])
            nc.vector.tensor_copy(qoT16[pt][:], qoT[pt][:])

        # V_in natural [t128 x 4, d256]
        vn = pa.tile([128, 4, D], F32)
        for tt in range(4):
            ps = pps.tile([128, L], F32, tag="ps")
            for ct in range(2):
                nc.tensor.matmul(
                    ps[:, 0:D], xT[ct][:, 128 * tt : 128 * (tt + 1)], wv_sb[ct][:],
                    start=(ct == 0), stop=(ct == 1),
                )
            nc.vector.tensor_copy(vn[:, tt, :], ps[:, 0:D])

        # K_slots^T [d'128 x 2, s128]
        kslT = [pa.tile([128, S], F32, name=f"kslT{pt}") for pt in range(2)]
        for pt in range(2):
            ps = pps.tile([128, L], F32, tag="ps")
            for ct in range(2):
                nc.tensor.matmul(
                    ps[:, 0:S], wksl_sb[ct][:, 128 * pt : 128 * (pt + 1)], hTt[ct][:],
                    start=(ct == 0), stop=(ct == 1),
                )
            nc.vector.tensor_copy(kslT[pt][:], ps[:, 0:S])

        # attention scores + masked softmax
        a_sb = pa.tile([128, 4, S], F32)
        for tt in range(4):
            sc = pps.tile([128, L], F32, tag="ps")
            for pt in range(2):
                nc.tensor.matmul(
                    sc[:, 0:S], qT[pt][:, 128 * tt : 128 * (tt + 1)], kslT[pt][:],
                    start=(pt == 0), stop=(pt == 1),
                )
            rowmax = pa.tile([128, 1], F32, tag="rmax")
            nc.vector.tensor_reduce(
                rowmax[:], sc[:, 0:S], axis=mybir.AxisListType.X,
                op=mybir.AluOpType.max,
            )
            nb = pa.tile([128, 1], F32, tag="nb")
            nc.vector.tensor_scalar_mul(nb[:], rowmax[:], -SCALE)
            sumexp = pa.tile([128, 1], F32, tag="sexp")
            nc.scalar.activation(
                a_sb[:, tt, :], sc[:, 0:S], mybir.ActivationFunctionType.Exp,
                bias=nb[:], scale=SCALE, accum_out=sumexp[:],
            )
            rs = pa.tile([128, 1], F32, tag="rs")
            nc.vector.reciprocal(rs[:], sumexp[:])
            rm = pa.tile([128, 1], F32, tag="rmk")
            nc.vector.tensor_tensor(
                rm[:], rs[:], mask_sb[:, tt : tt + 1], op=mybir.AluOpType.mult
            )
            nc.vector.tensor_scalar_mul(a_sb[:, tt, :], a_sb[:, tt, :], rm[:])

        # column sums and IR = A^T @ V
        cs = aps.tile([128, 1], F32, tag="cs")
        for tt in range(4):
            nc.tensor.matmul(
                cs[:], a_sb[:, tt, :], ones[:, 0:1], start=(tt == 0), stop=(tt == 3)
            )
        ir = aps.tile([128, D], F32, tag="ir")
        for tt in range(4):
            nc.tensor.matmul(
                ir[:], a_sb[:, tt, :], vn[:, tt, :], start=(tt == 0), stop=(tt == 3)
            )
        cssb = pa.tile([128, 1], F32)
        nc.vector.tensor_scalar_add(cssb[:], cs[:], 1e-8)
        rcs = pa.tile([128, 1], F32)
        nc.vector.reciprocal(rcs[:], cssb[:])
        h0 = pa.tile([S, D], F32)
        nc.vector.scalar_tensor_tensor(
            h0[:], ir[:], rcs[:], hn_sb[:],
            op0=mybir.AluOpType.mult, op1=mybir.AluOpType.add,
        )

        # h0 -> transposed bounce, init AllGather (fp32, 2 segments)
        agin0 = dra.tile([2 * 16384], F32)
        for dt in range(2):
            p3 = tps.tile([128, 128], F32, tag="tp")
            nc.tensor.transpose(p3[:], h0[:, 128 * dt : 128 * (dt + 1)], ident[:])
            h0T = pa.tile([128, 128], F32, tag="h0T")
            nc.vector.tensor_copy(h0T[:], p3[:])
            nc.sync.dma_start(
                agin0[dt * 16384 : (dt + 1) * 16384].rearrange(
                    "(p f) -> p f", p=128
                ),
                h0T[:],
            )
        agout0 = dra.tile([NC, 2 * 16384], F32, addr_space="Shared")
        nc.gpsimd.collective_compute(
            "AllGather", mybir.AluOpType.bypass,
            ins=[agin0[:].opt()], outs=[agout0[:].opt()],
            replica_groups=[list(range(NC))],
        )
        # readback: batch-major bounce (contiguous descriptors), then strided
        # casts into the [dp, s, b] layouts
        ag0r = agout0[:].rearrange(
            "b (seg dp s) -> seg dp b s", seg=2, dp=128, s=128
        )
        hA = [pa.tile([128, B, S], F32, name=f"hA{dt}") for dt in range(2)]
        tmp = pa.tile([128, B, JL], F32)
        for dt in range(2):
            nc.sync.dma_start(hA[dt][:], ag0r[dt])
            nc.vector.tensor_copy(
                hTh[dt][:], hA[dt][:].rearrange("p b s -> p s b")
            )
            nc.vector.tensor_copy(
                tmp[:], hA[dt][:, :, bass.ds(pid_v * JL, JL)]
            )
            nc.vector.tensor_copy(
                hTown[dt][:], tmp[:].rearrange("p b j -> p j b")
            )


def _step(nc, tc, t, io, hTh, hTown, lngbT_sb, ones, ones_row, eps_sb,
          wsp, wtp, isp, lnsb, p1p, inflp, lnps, bdram):
    """One message-passing step (fp16 weights, transposed-layout LN)."""
    # whole-step inter buffers: [p, il, g, jq, b] and regrouped [p, jloc, g, jq, b]
    # (il/jloc outermost so each regroup DMA moves 2KB-contiguous runs)
    inter = isp.tile([128, 4, G, 4, B], F16, tag="inter")
    inter2 = isp.tile([128, 4, G, 4, B], F16, tag="inter2")
    inflT = inflp.tile([128, 2, JL, B], F32, tag="inflT")

    # ---- einsum1: inter[(jloc,r), il, g, jq, b] ----
    for g in range(G):
        ws = wsp.tile([128, 4, 4, 2, 128], F16, tag="ws")
        nc.sync.dma_start(ws[:], io["wsrc16"][g])
        p1 = p1p.tile([128, 4, 4, B], F32, tag="p1")
        for il in range(4):
            i = 4 * g + il
            for jq in range(4):
                for dt in range(2):
                    nc.tensor.matmul(
                        p1[:, il, jq, :],
                        ws[:, il, jq, dt, :],
                        hTh[dt][:, i, :],
                        start=(dt == 0), stop=(dt == 1),
                    )
        nc.vector.tensor_copy(inter[:, :, g, :, :], p1[:])

    # ---- regroup: (jloc,r) bands -> (il,r) bands (SBUF->SBUF DMA) ----
    for il in range(4):
        for jloc in range(4):
            nc.sync.dma_start(
                inter2[32 * il : 32 * (il + 1), jloc],
                inter[32 * jloc : 32 * (jloc + 1), il],
            )

    # ---- einsum2: inflT[dc, dh, j, b] += inter2 @ W_target ----
    # PSUM start_tensor_calc marks the whole 2KB zero region (= the bank
    # holding all 32 (j, dh) accumulators) pending-zero, so exactly ONE
    # start on the first matmul and ONE stop on the last - a per-group
    # start would wipe the other groups' partial sums.
    for g in range(G):
        wt = wtp.tile([128, JL, 2, 128], F16, tag="wt")
        nc.scalar.dma_start(wt[:], io["wtgt16"][g])
        for j in range(JL):
            jq, jloc = j // 4, j % 4
            for dh in range(2):
                nc.tensor.matmul(
                    inflT[:, dh, j, :],
                    wt[:, j, dh, :],
                    inter2[:, jloc, g, jq, :],
                    start=(g == 0 and j == 0 and dh == 0),
                    stop=(g == G - 1 and j == JL - 1 and dh == 1),
                    skip_group_check=True,
                )

    # ---- relu + residual + LayerNorm, all in transposed layout ----
    hrelu = lnsb.tile([128, 2, JL, B], F32, tag="hrelu")
    nc.scalar.activation(hrelu[:], inflT[:], mybir.ActivationFunctionType.Relu)
    hsum = lnsb.tile([128, 2, JL, B], F32, tag="hsum")
    for dt in range(2):
        nc.vector.tensor_tensor(
            hsum[:, dt], hrelu[:, dt], hTown[dt][:], op=mybir.AluOpType.add
        )
    sq = lnsb.tile([128, 2, JL, B], F32, tag="sq")
    nc.vector.tensor_tensor(sq[:], hsum[:], hsum[:], op=mybir.AluOpType.mult)
    sums = lnps.tile([1, 2, 2, JL, B], F32, tag="sums")  # [1, (s/sq), dt, j, b]
    nc.tensor.matmul(
        sums[0:1, 0], ones[:, 0:1], hsum[:], start=True, stop=True,
    )
    nc.tensor.matmul(
        sums[0:1, 1], ones[:, 0:1], sq[:], start=True, stop=True,
    )
    # mean/rstd per (j, b): combine dt halves on 1 partition
    sums_sb = lnsb.tile([1, 2, 2, JL, B], F32, tag="sums_sb")
    nc.vector.tensor_copy(sums_sb[:], sums[:])
    mrs = lnsb.tile([1, 2, JL * B], F32, tag="mrs")  # [1, (mean, rstd), jb]
    mean = mrs[0:1, 0]
    nc.vector.tensor_tensor(
        mean, sums_sb[0:1, 0, 0].rearrange("p j b -> p (j b)"),
        sums_sb[0:1, 0, 1].rearrange("p j b -> p (j b)"), op=mybir.AluOpType.add,
    )
    nc.vector.tensor_scalar_mul(mean, mean, 1.0 / D)
    ssq = lnsb.tile([1, JL * B], F32, tag="ssq")
    nc.vector.tensor_tensor(
        ssq[:], sums_sb[0:1, 1, 0].rearrange("p j b -> p (j b)"),
        sums_sb[0:1, 1, 1].rearrange("p j b -> p (j b)"), op=mybir.AluOpType.add,
    )
    nc.vector.tensor_scalar_mul(ssq[:], ssq[:], 1.0 / D)
    msq = lnsb.tile([1, JL * B], F32, tag="msq")
    nc.vector.tensor_tensor(msq[:], mean, mean, op=mybir.AluOpType.mult)
    var = lnsb.tile([1, JL * B], F32, tag="var")
    nc.vector.tensor_tensor(var[:], ssq[:], msq[:], op=mybir.AluOpType.subtract)
    std = lnsb.tile([1, JL * B], F32, tag="std")
    nc.scalar.activation(
        std[:], var[:], mybir.ActivationFunctionType.Sqrt, bias=eps_sb[0:1, :]
    )
    rstd = mrs[0:1, 1]
    nc.vector.reciprocal(rstd, std[:])
    # broadcast mean/rstd across partitions via 1-row ones matmul
    bc = lnps.tile([128, 2, JL * B], F32, tag="bc")
    nc.tensor.matmul(
        bc[:], ones_row[:], mrs[0:1].rearrange("p m jb -> p (m jb)"),
        start=True, stop=True,
    )
    # normalize + affine; write fp32 own-state and fp16 gather input
    hnewTh = lnsb.tile([128, 2, JL, B], F16, tag="hnewTh")
    cen = lnsb.tile([128, JL, B], F32, tag="cen")
    for dt in range(2):
        nc.vector.tensor_tensor(
            cen[:], hsum[:, dt],
            bc[:, 0].rearrange("p (j b) -> p j b", j=JL),
            op=mybir.AluOpType.subtract,
        )
        nc.vector.tensor_tensor(
            cen[:], cen[:],
            bc[:, 1].rearrange("p (j b) -> p j b", j=JL),
            op=mybir.AluOpType.mult,
        )
        nc.vector.tensor_scalar(
            hTown[dt][:], cen[:],
            lngbT_sb[:, t, dt : dt + 1],
            lngbT_sb[:, t, 2 + dt : 3 + dt],
            op0=mybir.AluOpType.mult, op1=mybir.AluOpType.add,
        )
        nc.vector.tensor_copy(hnewTh[:, dt], hTown[dt][:])

    # ---- AllGather the 16 updated slots (fp16) ----
    agin = bdram.tile([128 * 2 * JL * B], F16, tag="agin")
    nc.gpsimd.dma_start(
        agin[:].rearrange("(p f) -> p f", p=128), hnewTh[:]
    )
    agout = bdram.tile([NC, 128 * 2 * JL * B], F16, addr_space="Shared",
                       tag="agout")
    nc.gpsimd.collective_compute(
        "AllGather", mybir.AluOpType.bypass,
        ins=[agin[:].opt()], outs=[agout[:].opt()],
        replica_groups=[list(range(NC))],
    )
    agr = agout[:].rearrange(
        "k (dp dt jl b) -> dt dp k jl b", dp=128, dt=2, jl=JL, b=B
    )
    for dt in range(2):
        nc.gpsimd.dma_start(
            hTh[dt][:].rearrange("dp (k jl) b -> dp k jl b", k=NC), agr[dt]
        )

    if DBG and t == 0:
        dbg = io["dbg"]
        for dt in range(2):
            nc.sync.dma_start(
                dbg[:, 128 * dt : 128 * (dt + 1)].rearrange(
                    "p (j b) -> p j b", j=JL
                ),
                hTown[dt][:],
            )
        nc.sync.dma_start(
            dbg[:, 256:512].rearrange("p (d j b) -> p d j b", d=2, j=JL),
            hrelu[:],
        )
        nc.sync.dma_start(
            dbg[:, 512:768].rearrange("p (d j b) -> p d j b", d=2, j=JL),
            hsum[:],
        )
        nc.sync.dma_start(
            dbg[0:1, 1024:1280].rearrange("p (m jb) -> p m jb", m=2), mrs[:]
        )
        nc.gpsimd.dma_start(
            dbg[:, 2048:2560].rearrange("p (il g jq b) -> p il g jq b",
                                        g=4, il=4, jq=4),
            inter[:, :, 0:4],
        )
        nc.gpsimd.dma_start(
            dbg[:, 2560:3072].rearrange("p (jl g jq b) -> p jl g jq b",
                                        g=4, jl=4, jq=4),
            inter2[:, :, 0:4],
        )
        nc.gpsimd.dma_start(
            dbg[:, 3072:4096].rearrange("p (s b) -> p s b", s=S),
            hTh[0][:],
        )


def _phase_c(nc, tc, io, ident, pid_v, hTh, qoT16):
    with tc.tile_pool(name="pc_sb", bufs=1) as pc, \
         tc.tile_pool(name="pc_ps", bufs=3, space="PSUM") as cps, \
         tc.tile_pool(name="pc_lg", bufs=4, space="PSUM") as lgps, \
         tc.tile_pool(name="pc_wo", bufs=3) as wop, \
         tc.tile_pool(name="pc_lgsb", bufs=3) as lgsb:

        wkf_sb = pc.tile([128, 2, D], F16)
        wvf_sb = pc.tile([128, 2, D], F16)
        for ct in range(2):
            nc.sync.dma_start(
                wkf_sb[:, ct], io["wkf16"][128 * ct : 128 * (ct + 1), :]
            )
            nc.sync.dma_start(
                wvf_sb[:, ct], io["wvf16"][128 * ct : 128 * (ct + 1), :]
            )

        # own-batch h^T slice (dynamic b=pid) -> static tiles
        hb = [pc.tile([128, S], F16, name=f"hb{dt}") for dt in range(2)]
        for dt in range(2):
            nc.vector.tensor_copy(
                hb[dt][:].rearrange("p (s o) -> p s o", o=1),
                hTh[dt][:, :, bass.ds(pid_v, 1)],
            )

        # K_f^T [d'128 x2, s128] ; V_f natural [s, d'] (fp16)
        kfT = [pc.tile([128, S], F16, name=f"kfT{pt}") for pt in range(2)]
        for pt in range(2):
            ps = cps.tile([128, L], F32, tag="c")
            for ct in range(2):
                nc.tensor.matmul(
                    ps[:, 0:S], wkf_sb[:, ct, 128 * pt : 128 * (pt + 1)],
                    hb[ct][:],
                    start=(ct == 0), stop=(ct == 1),
                )
            nc.vector.tensor_copy(kfT[pt][:], ps[:, 0:S])
        vf = pc.tile([S, D], F16)
        psv = cps.tile([128, L], F32, tag="c")
        for ct in range(2):
            nc.tensor.matmul(
                psv[0:S, 0:D], hb[ct][:], wvf_sb[:, ct],
                start=(ct == 0), stop=(ct == 1),
            )
        nc.vector.tensor_copy(vf[:], psv[0:S, 0:D])

        # expand attention -> A2^T [s, t512] (fp16)
        a2T = pc.tile([S, L], F16)
        for tt in range(4):
            sc = cps.tile([128, L], F32, tag="c")
            for pt in range(2):
                nc.tensor.matmul(
                    sc[:, 0:S], qoT16[pt][:, 128 * tt : 128 * (tt + 1)], kfT[pt][:],
                    start=(pt == 0), stop=(pt == 1),
                )
            rowmax = pc.tile([128, 1], F32, tag="rmax2")
            nc.vector.tensor_reduce(
                rowmax[:], sc[:, 0:S], axis=mybir.AxisListType.X,
                op=mybir.AluOpType.max,
            )
            nb = pc.tile([128, 1], F32, tag="nb2")
            nc.vector.tensor_scalar_mul(nb[:], rowmax[:], -SCALE)
            a2 = pc.tile([128, S], F32, tag="a2")
            sumexp = pc.tile([128, 1], F32, tag="sexp2")
            nc.scalar.activation(
                a2[:], sc[:, 0:S], mybir.ActivationFunctionType.Exp,
                bias=nb[:], scale=SCALE, accum_out=sumexp[:],
            )
            rs = pc.tile([128, 1], F32, tag="rs2")
            nc.vector.reciprocal(rs[:], sumexp[:])
            nc.vector.tensor_scalar_mul(a2[:], a2[:], rs[:])
            ptr = cps.tile([128, L], F32, tag="c")
            nc.tensor.transpose(ptr[:, 0:S], a2[:], ident[:])
            nc.vector.tensor_copy(a2T[:, 128 * tt : 128 * (tt + 1)], ptr[:, 0:S])

        # Y^T [d128 x2, t512] (fp16)
        yT = [pc.tile([128, L], F16, name=f"yT{dt}") for dt in range(2)]
        for dt in range(2):
            ps = cps.tile([128, L], F32, tag="c")
            nc.tensor.matmul(
                ps[:], vf[:, 128 * dt : 128 * (dt + 1)], a2T[:],
                start=True, stop=True,
            )
            nc.vector.tensor_copy(yT[dt][:], ps[:])

        # logits: out[t128, v500] per (vt, tt); fp16 store batched per vt pair
        for vp in range(NVT // 2):
            lg_sb = lgsb.tile([128, 4, 2, VT], F16, tag="lg_sb")
            for vh in range(2):
                vt = 2 * vp + vh
                wo_sb = wop.tile([128, 2, VT], F16, tag="wo")
                nc.scalar.dma_start(wo_sb[:], io["wout16"][vt])
                for tt in range(4):
                    lg = lgps.tile([128, VT], F32, tag="lg")
                    for dt in range(2):
                        nc.tensor.matmul(
                            lg[:],
                            yT[dt][:, 128 * tt : 128 * (tt + 1)],
                            wo_sb[:, dt, :],
                            start=(dt == 0), stop=(dt == 1),
                        )
                    nc.any.tensor_copy(lg_sb[:, tt, vh, :], lg[:])
            nc.sync.dma_start(
                io["lg_out"].rearrange("(tt p) v -> p tt v", tt=4)[
                    :, :, 2 * VT * vp : 2 * VT * (vp + 1)
                ].rearrange("p tt (vh v) -> p tt vh v", vh=2),
                lg_sb[:],
            )


# ---------------------------------------------------------------------------
# Host side
# ---------------------------------------------------------------------------

_NC_CACHE = {}


def _get_nc():
    key = N_STEPS
    if key not in _NC_CACHE:
        _NC_CACHE[key] = build()
    return _NC_CACHE[key]


def _prep_in_maps(inputs):
    f32 = lambda a: np.ascontiguousarray(np.asarray(a), dtype=np.float32)
    input_ids = np.asarray(inputs["input_ids"])
    attention_mask = np.asarray(inputs["attention_mask"])
    H = f32(inputs["H"])
    W_source = np.asarray(inputs["W_source"], dtype=np.float32)
    W_target = np.asarray(inputs["W_target"], dtype=np.float32)

    # lngbT[p, t, 0:2] = gamma[t, 128*dt+p]; [p, t, 2:4] = beta
    lngbT = np.zeros((128, STEPS, 4), dtype=np.float32)
    lnsc = np.asarray(inputs["ln_scale"], dtype=np.float32).reshape(STEPS, 2, 128)
    lnbi = np.asarray(inputs["ln_bias"], dtype=np.float32).reshape(STEPS, 2, 128)
    lngbT[:, :, 0:2] = lnsc.transpose(2, 0, 1)
    lngbT[:, :, 2:4] = lnbi.transpose(2, 0, 1)

    rep = {
        "wqT": f32(np.asarray(inputs["Wq_in"]).T),
        "wkslT": f32(np.asarray(inputs["Wk_slots"]).T),
        "wvT": f32(np.asarray(inputs["Wv_in"]).T),
        "wqoT": f32(np.asarray(inputs["Wq_out"]).T),
        "wkf16": np.ascontiguousarray(
            np.asarray(inputs["Wk_fin"]).T.astype(np.float16)
        ),
        "wvf16": np.ascontiguousarray(
            np.asarray(inputs["Wv_fin"]).T.astype(np.float16)
        ),
        "hT_in": f32(H.T),
        "hn_in": H,
        "lngbT": lngbT,
        # wout16[vt, dp, dt, v] = Wout[500vt+v, 128dt+dp]
        "wout16": np.ascontiguousarray(
            f32(inputs["W_out_proj"]).reshape(NVT, VT, 2, 128)
            .transpose(0, 3, 2, 1).astype(np.float16)
        ),
    }

    in_maps = []
    for c in range(NC):
        m = dict(rep)
        X = (np.asarray(inputs["token_emb"], dtype=np.float32)[input_ids[c]]
             + np.asarray(inputs["pos_emb"], dtype=np.float32))
        m["xT_in"] = np.ascontiguousarray(X.T)
        m["maskw"] = np.ascontiguousarray(
            attention_mask[c].astype(np.float32).reshape(4, 128).T
        )
        # wsrc16[g, dp, il, jq, dt, (jloc r)] = Ws[4g+il, 16c+4jq+jloc, 128dt+dp, r]
        ws = W_source[:, JL * c : JL * (c + 1)]      # [128 i, 16 j, 256 d, 32 r]
        ws = ws.reshape(G, 4, 4, 4, 2, 128, R)       # [g, il, jq, jloc, dt, dp, r]
        ws = ws.transpose(0, 5, 1, 2, 4, 3, 6)       # [g, dp, il, jq, dt, jloc, r]
        m["wsrc16"] = np.ascontiguousarray(
            ws.reshape(G, 128, 4, 4, 2, 128).astype(np.float16)
        )
        # wtgt16[g, (il r), j, dh, dc] = Wt[4g+il, 16c+j, r, 128dh+dc], diag zeroed
        wt = W_target[:, JL * c : JL * (c + 1)].copy()   # [128 i, 16 j, 32 r, 256 d]
        for jl in range(JL):
            wt[JL * c + jl, jl] = 0.0
        wt = wt.reshape(G, 4, JL, R, 2, 128)         # [g, il, j, r, dh, dc]
        wt = wt.transpose(0, 1, 3, 2, 4, 5)          # [g, il, r, j, dh, dc]
        m["wtgt16"] = np.ascontiguousarray(
            wt.reshape(G, 128, JL, 2, 128).astype(np.float16)
        )
        in_maps.append(m)
    return in_maps


def run(inputs, trace=False):
    nc = _get_nc()
    in_maps = _prep_in_maps(inputs)
    res = bass_utils.run_bass_kernel_spmd(
        nc, in_maps, core_ids=list(range(NC)), trace=trace
    )
    out = np.stack(
        [res.results[c]["lg_out"].astype(np.float32) for c in range(NC)], axis=0
    )
    return out, res


def kernel(**inputs):
    out, _ = run(inputs, trace=False)
    return out
